# revision 32
# baseline (speedup 1.0000x reference)
"""Trainium2 Bass kernel for quantized BasicBlock (DoReFa conv-bn-act x2 + residual).

Self-contained: builds an 8-core SPMD Bass kernel, shards the batch (64 -> 8x8),
runs via bass_utils.run_bass_kernel_spmd, gathers the full output.

Math (per core, batch shard of 8 images):
  W_int = 2*rint(tanh(w)*s + 7.5) - 15, s = 15/(2*max|tanh(w)|)   (odd ints, |.|<=15)
  conv1: S1 = conv3x3(fp16(x), W1_int)      == 15 * conv3x3(x, w_q1) + eps_fp16
  BN1 stats of S1 over (N,H,W) all-reduced across cores (split 6+2 images so the
  first AllReduce hides under the tail of conv1)
  act1  = min(rint(relu(S1*sc1 + bi1)), 15)  (ints 0..15, stored fp8e4m3)
  conv2: S2 = conv3x3(act1, W2_int)          == 225 * conv3x3(a_q, w_q2), exact
         (fp8 matmuls; dx-adjacent tap pairs fused via DoubleRow perf mode)
  BN2 stats of S2 all-reduced (same 6+2 split)
  tail : PSUM = diag(15*sc2)@S2 + (15*I)@x + diag(15*bi2)@ones   (f32r matmuls)
         out  = (clip(rint(PSUM), 0, 15)) / 15
"""
import sys
from contextlib import ExitStack

import numpy as np

for _p in ("/opt/trn_rl_repo",):
    if _p not in sys.path:
        sys.path.append(_p)

import concourse.bass as bass
import concourse.bass_isa as bass_isa
import concourse.bacc as bacc
import concourse.mybir as mybir
import concourse.tile as tile
from concourse import bass_utils
from concourse.bass import AP
from concourse.masks import make_identity

F32 = mybir.dt.float32
F32R = mybir.dt.float32r
FP16 = mybir.dt.float16
FP8 = mybir.dt.float8e4

N_CORES = 8
B, C, H, W = 64, 128, 56, 56
BPC = B // N_CORES            # images per core
HP, WP = H + 2, W + 2         # padded 58x58
PW = HP * WP                  # 3364
HW = H * W                    # 3136
RPT = 8                       # output rows per PSUM tile
TN = RPT * W                  # 448 columns per matmul
TPI = H // RPT                # 7 tiles per image
PSTRIDE = 512                 # PSUM bank stride in f32 elements
C23 = float(2 ** 23)
K1 = 15.0                     # conv1 PSUM = 15 * true conv
K2 = 225.0                    # conv2 PSUM = 225 * true conv
N_A = 4                       # images in the first (hidden) stats AllReduce
ROWS_A = 33                   # x rows feeding conv chunk A (+1 halo overlap)
USE_DR = True                 # DoubleRow fp8 pairing for conv2

TAPS = [(dy, dx) for dy in range(3) for dx in range(3)]

# conv PSUM chunks: (first tile, n tiles). 4+3 tiles -> 4+3 banks, 8th bank for
# the weight transposes.
CHUNKS = [(0, 4), (4, 3)]

_CACHE = {}


def _quant_weights(nc, pools, w_in, identity, ones_row, name):
    """DMA + DoReFa-quantize weights in-place on one (C, C*9) f32 tile.

    The cross-partition absmax runs on PE/DVE (transpose -> free-axis reduce
    -> matmul broadcast) instead of gpsimd: the Q7 custom-op launch costs
    ~15us on the critical path.
    """
    wp = pools["wprep"]
    trp = pools["psT"]
    wk = pools["stage"].tile([C, C * 9], F32, name=f"{name}_wk", tag="stage")
    half = C * 9 // 2
    nc.scalar.dma_start(wk[:, 0:half], w_in[:, 0:half])
    nc.scalar.dma_start(wk[:, half:], w_in[:, half:])
    am = wp.tile([C, 1], F32, name=f"{name}_am", tag="wam")
    nc.vector.tensor_reduce(am[:], wk[:], axis=mybir.AxisListType.X,
                            op=mybir.AluOpType.max, apply_absolute_value=True)
    nc.scalar.activation(wk[:], wk[:], mybir.ActivationFunctionType.Tanh)
    # partition max: transpose [C,1] -> [1,C], reduce on one lane, broadcast
    psr = trp.tile([C, C], F32, name=f"{name}_psr", tag="trps")
    nc.tensor.transpose(psr[0:1, 0:C], am[:], identity[:])
    amr = wp.tile([1, C], F32, name=f"{name}_amr", tag="wamr")
    nc.scalar.copy(amr[:], psr[0:1, 0:C])
    am0 = wp.tile([1, 1], F32, name=f"{name}_am0", tag="wam0")
    nc.vector.tensor_reduce(am0[:], amr[:], axis=mybir.AxisListType.X,
                            op=mybir.AluOpType.max)
    psb = trp.tile([C, C], F32, name=f"{name}_psb", tag="trps")
    nc.tensor.matmul(psb[0:C, 0:1], ones_row[:], am0[:], start=True, stop=True)
    amg = wp.tile([C, 1], F32, name=f"{name}_amg", tag="wamg")
    nc.scalar.copy(amg[:], psb[0:C, 0:1])
    s_t = wp.tile([C, 1], F32, name=f"{name}_s", tag="ws")
    nc.scalar.activation(s_t[:], amg[:], mybir.ActivationFunctionType.Tanh)
    nc.vector.reciprocal(s_t[:], s_t[:])
    nc.vector.tensor_scalar_mul(s_t[:], s_t[:], 7.5)
    # W_int = 2*rint(tanh*s + 7.5) - 15
    nc.vector.tensor_scalar(wk[:], wk[:], s_t[:], 7.5,
                            op0=mybir.AluOpType.mult, op1=mybir.AluOpType.add)
    nc.vector.tensor_scalar(wk[:], wk[:], C23, C23,
                            op0=mybir.AluOpType.add, op1=mybir.AluOpType.subtract)
    nc.vector.tensor_scalar(wk[:], wk[:], 2.0, 15.0,
                            op0=mybir.AluOpType.mult, op1=mybir.AluOpType.subtract)
    return wk


def _transpose_taps(nc, pools, wint, identity, out_dt, name):
    """Per-tap PE transpose of W_int (O,(I,t)) -> wT (I,(t,O)) in out_dt."""
    wp = pools["wconst"]
    trp = pools["psT"]
    wT = wp.tile([C, 9 * C], out_dt, name=f"{name}_T")
    wr = wint.rearrange("p (i t) -> p i t", t=9)
    for t in range(9):
        ps = trp.tile([C, C], F32, name=f"{name}_ps{t}", tag="trps")
        nc.tensor.transpose(ps[:], wr[:, :, t], identity[:])
        nc.scalar.copy(wT[:, t * C:(t + 1) * C], ps[:])
    return wT


def _warmup_allreduce_eps(nc, pools):
    """Tiny AllReduce at kernel start: warms up ncfw and produces the BN
    epsilon constant (8 * 1e-5/8) so it survives DCE."""
    sp = pools["stats"]
    dp = pools["dram"]
    eps8 = sp.tile([C, 1], F32, name="eps8")
    nc.gpsimd.memset(eps8[:], 1e-5 / N_CORES)
    cc_in = dp.tile([C, 1], F32, name="ccw_in")
    cc_out = dp.tile([C, 1], F32, name="ccw_out")
    nc.gpsimd.dma_start(cc_in[:], eps8[:])
    nc.gpsimd.collective_compute(
        "AllReduce", mybir.AluOpType.add,
        replica_groups=[list(range(N_CORES))],
        ins=[cc_in.opt()], outs=[cc_out.opt()],
    )
    epst = sp.tile([C, 1], F32, name="epst")
    nc.sync.dma_start(epst[:], cc_out[:])
    return epst


def _stats_payload_ar(nc, pools, stats, i0, i1, k_scale, name):
    """bn_aggr over images [i0, i1) -> payload (sum_x, sum_x2)/N_global in
    UNSCALED units -> AllReduce. Returns the DRAM tile holding the result."""
    sp = pools["stats"]
    dp = pools["dram"]
    nimg = i1 - i0
    wfrac = float(nimg) / float(B)
    loc = sp.tile([C, 2], F32, name=f"{name}_loc")
    nc.vector.bn_aggr(loc[:], stats[:, i0 * TPI * 6:i1 * TPI * 6].rearrange(
        "p (t k) -> p t k", k=3))
    mu = sp.tile([C, 1], F32, name=f"{name}_mu")
    nc.vector.tensor_scalar_mul(mu[:], loc[:, 0:1], 1.0 / k_scale)
    pay = sp.tile([C, 2], F32, name=f"{name}_pay")
    nc.vector.tensor_scalar_mul(pay[:, 0:1], mu[:], wfrac)
    # pay1 = wfrac * (var/k^2 + mu^2)
    msq = sp.tile([C, 1], F32, name=f"{name}_msq")
    nc.vector.scalar_tensor_tensor(msq[:], mu[:], wfrac, mu[:],
                                   op0=mybir.AluOpType.mult,
                                   op1=mybir.AluOpType.mult)
    nc.vector.scalar_tensor_tensor(pay[:, 1:2], loc[:, 1:2],
                                   wfrac / (k_scale * k_scale), msq[:],
                                   op0=mybir.AluOpType.mult,
                                   op1=mybir.AluOpType.add)
    cc_in = dp.tile([C, 2], F32, name=f"{name}_in")
    cc_out = dp.tile([C, 2], F32, name=f"{name}_out")
    nc.sync.dma_start(cc_in[:], pay[:])
    nc.gpsimd.collective_compute(
        "AllReduce", mybir.AluOpType.add,
        replica_groups=[list(range(N_CORES))],
        ins=[cc_in.opt()], outs=[cc_out.opt()],
    )
    return cc_out


def _combine_stats(nc, pools, ccA, ccB, epst, name):
    """Fetch both AllReduce results, combine -> (mean_u, rstd_u)."""
    sp = pools["stats"]
    gA = sp.tile([C, 2], F32, name=f"{name}_gA")
    gB = sp.tile([C, 2], F32, name=f"{name}_gB")
    nc.sync.dma_start(gA[:], ccA[:])
    nc.sync.dma_start(gB[:], ccB[:])
    gs = sp.tile([C, 2], F32, name=f"{name}_gs")
    nc.vector.tensor_tensor(gs[:], gA[:], gB[:], op=mybir.AluOpType.add)
    mean_g = gs[:, 0:1]
    m2 = sp.tile([C, 1], F32, name=f"{name}_m2")
    nc.vector.scalar_tensor_tensor(m2[:], mean_g, 1.0, mean_g,
                                   op0=mybir.AluOpType.mult,
                                   op1=mybir.AluOpType.mult)
    varg = sp.tile([C, 1], F32, name=f"{name}_var")
    nc.vector.scalar_tensor_tensor(varg[:], m2[:], -1.0, gs[:, 1:2],
                                   op0=mybir.AluOpType.mult,
                                   op1=mybir.AluOpType.add)
    std = sp.tile([C, 1], F32, name=f"{name}_std")
    nc.scalar.activation(std[:], varg[:], mybir.ActivationFunctionType.Sqrt,
                         bias=epst[:])
    rstd = sp.tile([C, 1], F32, name=f"{name}_rstd")
    nc.vector.reciprocal(rstd[:], std[:])
    return mean_g, rstd


def _affine_vecs(nc, pools, gamma, beta, mean_u, rstd_u, m_out, k_scale, name):
    """For y_out = m*bn(S/k): sc = m*gamma*rstd/k ; bi = m*(beta - mean_u*gamma*rstd)."""
    sp = pools["stats"]
    gr = sp.tile([C, 1], F32, name=f"gr{name}")
    nc.vector.scalar_tensor_tensor(gr[:], gamma[:], 1.0, rstd_u[:],
                                   op0=mybir.AluOpType.bypass,
                                   op1=mybir.AluOpType.mult)
    sc = sp.tile([C, 1], F32, name=f"sc{name}")
    nc.vector.tensor_scalar_mul(sc[:], gr[:], m_out / k_scale)
    negms = sp.tile([C, 1], F32, name=f"negms{name}")
    nc.vector.scalar_tensor_tensor(negms[:], mean_u, -1.0, gr[:],
                                   op0=mybir.AluOpType.mult,
                                   op1=mybir.AluOpType.mult)
    bi = sp.tile([C, 1], F32, name=f"bi{name}")
    nc.vector.scalar_tensor_tensor(bi[:], negms[:], 1.0, beta[:],
                                   op0=mybir.AluOpType.bypass,
                                   op1=mybir.AluOpType.add)
    nc.vector.tensor_scalar_mul(bi[:], bi[:], m_out)
    return sc, bi


def _dr_rhs(img_view, t, dy):
    """DoubleRow rhs: overlapping 4D AP [C, 2, RPT, W]; pair dim = dx 0/1
    (stride 1 fp8 element)."""
    base = img_view[:, RPT * t + dy: RPT * t + dy + RPT, 0:W]
    u = base.unsqueeze(1)
    ap = [list(p) for p in u.ap]
    ap[1] = [1, 2]
    return AP(u.tensor, u.offset, ap)


def _conv_image(nc, pools, wT, img_view, out_sb, use_dr):
    """One image: 2 PSUM chunks; per tile accumulate 9 taps; per chunk a single
    strided ACT copy-out PSUM -> out_sb."""
    for ci, (t0, ntil) in enumerate(CHUNKS):
        pool = pools["psA" if ci == 0 else "psB"]
        ps = pool.tile([C, PSTRIDE * ntil], F32, name=f"cv{ci}",
                       tag=f"cvch{ci}")
        for i in range(ntil):
            t = t0 + i
            sl = ps[:, i * PSTRIDE:i * PSTRIDE + TN]
            if use_dr:
                for dy in range(3):
                    lhsT = wT[:, (3 * dy) * C:(3 * dy + 2) * C].rearrange(
                        "p (two f) -> p two f", two=2)
                    nc.tensor.matmul(sl, lhsT, _dr_rhs(img_view, t, dy),
                                     start=(dy == 0), stop=False,
                                     perf_mode=mybir.MatmulPerfMode.DoubleRow)
                for dy in range(3):
                    k = 3 * dy + 2
                    rhs = img_view[:, RPT * t + dy: RPT * t + dy + RPT, 2:2 + W]
                    nc.tensor.matmul(sl, wT[:, k * C:(k + 1) * C], rhs,
                                     start=False, stop=(dy == 2))
            else:
                for k, (dy, dx) in enumerate(TAPS):
                    rhs = img_view[:, RPT * t + dy: RPT * t + dy + RPT,
                                   dx: dx + W]
                    nc.tensor.matmul(sl, wT[:, k * C:(k + 1) * C], rhs,
                                     start=(k == 0), stop=(k == 8))
        # single strided copy-out for the chunk
        src = ps.rearrange("p (t c) -> p t c", c=PSTRIDE)[:, :, 0:TN]
        dst = out_sb[:, t0 * TN:(t0 + ntil) * TN].rearrange(
            "p (t c) -> p t c", c=TN)
        nc.scalar.copy(dst, src)


def _zero_halo(nc, xb, dt_zero=0.0):
    """Zero the 1-px halo of a padded [C, PW] image tile (3 memsets)."""
    xbr = xb.rearrange("p (h w) -> p h w", w=WP)
    nc.gpsimd.memset(xbr[:, 0, :], dt_zero)
    nc.gpsimd.memset(xbr[:, HP - 1, :], dt_zero)
    side = xb[:, WP - 1:WP - 1 + (HP - 1) * WP].rearrange(
        "p (a b) -> p a b", b=WP)
    nc.gpsimd.memset(side[:, :, 0:2], dt_zero)


def build():
    nc = bacc.Bacc("TRN2", target_bir_lowering=False, debug=False,
                   enable_asserts=False, num_devices=N_CORES)
    x_in = nc.dram_tensor("x", [BPC, C, H, W], F32, kind="ExternalInput").ap()
    w1_in = nc.dram_tensor("w1", [C, C * 9], F32, kind="ExternalInput").ap()
    w2_in = nc.dram_tensor("w2", [C, C * 9], F32, kind="ExternalInput").ap()
    g1_in = nc.dram_tensor("gamma1", [C, 1], F32, kind="ExternalInput").ap()
    b1_in = nc.dram_tensor("beta1", [C, 1], F32, kind="ExternalInput").ap()
    g2_in = nc.dram_tensor("gamma2", [C, 1], F32, kind="ExternalInput").ap()
    b2_in = nc.dram_tensor("beta2", [C, 1], F32, kind="ExternalInput").ap()
    out_d = nc.dram_tensor("out", [BPC, C, H, W], F32, kind="ExternalOutput").ap()

    with tile.TileContext(nc) as tc, ExitStack() as ctx:
        pools = {
            "wprep": ctx.enter_context(tc.tile_pool(name="wprep", bufs=1)),
            "wconst": ctx.enter_context(tc.tile_pool(name="wconst", bufs=1)),
            "stats": ctx.enter_context(tc.tile_pool(name="stats", bufs=1)),
            "xp16": ctx.enter_context(tc.tile_pool(name="xp16", bufs=8)),
            "big": ctx.enter_context(tc.tile_pool(name="big", bufs=8)),
            "a1": ctx.enter_context(tc.tile_pool(name="a1", bufs=2)),
            # shared staging ring: weight-quant scratch, x fp32 staging, and
            # tail result buffers all rotate through 3 slots
            "stage": ctx.enter_context(tc.tile_pool(name="stage", bufs=3)),
            "psA": ctx.enter_context(
                tc.tile_pool(name="psA", bufs=1, space="PSUM")),
            "psB": ctx.enter_context(
                tc.tile_pool(name="psB", bufs=1, space="PSUM")),
            "psT": ctx.enter_context(
                tc.tile_pool(name="psT", bufs=1, space="PSUM")),
            "dram": ctx.enter_context(tc.tile_pool(name="dram", bufs=12,
                                                   space="DRAM")),
        }
        consts = pools["wconst"]

        # per-channel params (sync queue; scalar queue is loading w1)
        g1 = consts.tile([C, 1], F32, name="g1"); nc.sync.dma_start(g1[:], g1_in[:])
        b1 = consts.tile([C, 1], F32, name="b1"); nc.sync.dma_start(b1[:], b1_in[:])
        g2 = consts.tile([C, 1], F32, name="g2"); nc.sync.dma_start(g2[:], g2_in[:])
        b2 = consts.tile([C, 1], F32, name="b2"); nc.sync.dma_start(b2[:], b2_in[:])

        identity = consts.tile([C, C], F32, name="identity")
        make_identity(nc, identity[:])
        # fp16 identity*15 and ones for the tail matmuls (fp16 weights keep
        # FWL weight loads fast); the diag(sc2) stays f32r for precision
        i15 = consts.tile([C, C], FP16, name="i15")
        nc.vector.tensor_scalar_mul(i15[:], identity[:], 15.0)
        ones = consts.tile([C, TN], FP16, name="ones")
        nc.vector.memset(ones[:], 1.0)
        ones_row = consts.tile([1, C], F32, name="ones_row")
        nc.vector.memset(ones_row[:], 1.0)

        epst = _warmup_allreduce_eps(nc, pools)

        # ---- w1 quant + transpose (critical path to first conv MM) ----
        w1i = _quant_weights(nc, pools, w1_in, identity, ones_row, "w1")
        w1T = _transpose_taps(nc, pools, w1i, identity, FP16, "w1")

        # ---- phase A: conv1 per image (single fp16 pass) ----
        stats1 = pools["stats"].tile([C, BPC * TPI * 6], F32, name="stats1")
        out1 = []
        cc1A = None
        w2T = None
        xp16s = []
        for n in range(BPC):
            # x staged fp32 (sync DMA, two halves) then ACT-converted into the
            # padded fp16 image; conv chunk A only depends on the first 33
            # rows (region-granular deps). The fp16 copy also serves as the
            # tail's residual (no reload).
            xs = pools["stage"].tile([C, HW], F32, name=f"xs{n}", tag="stage")
            xin = x_in[n].rearrange("c h w -> c (h w)")
            nc.sync.dma_start(xs[:, 0:ROWS_A * W], xin[:, 0:ROWS_A * W])
            nc.sync.dma_start(xs[:, ROWS_A * W:], xin[:, ROWS_A * W:])
            xp = pools["xp16"].tile([C, PW], FP16, name=f"xp{n}", tag="xp")
            _zero_halo(nc, xp)
            xpr = xp.rearrange("p (h w) -> p h w", w=WP)
            xsr = xs.rearrange("p (h w) -> p h w", w=W)
            nc.scalar.activation(xpr[:, 1:1 + ROWS_A, 1:1 + W],
                                 xsr[:, 0:ROWS_A, :],
                                 mybir.ActivationFunctionType.Copy)
            nc.scalar.activation(xpr[:, 1 + ROWS_A:1 + H, 1:1 + W],
                                 xsr[:, ROWS_A:H, :],
                                 mybir.ActivationFunctionType.Copy)
            xp16s.append(xp)
            o1 = pools["big"].tile([C, HW], F32, name=f"o1_{n}", tag="bigbuf")
            _conv_image(nc, pools, w1T, xpr, o1, use_dr=False)
            for t in range(TPI):
                gi = n * TPI + t
                nc.vector.bn_stats(stats1[:, gi * 6:(gi + 1) * 6],
                                   o1[:, t * TN:(t + 1) * TN])
            out1.append(o1)
            if n == 0:
                # w2 prep hides under conv1 of images 1..7
                w2i = _quant_weights(nc, pools, w2_in, identity, ones_row, "w2")
                w2T = _transpose_taps(nc, pools, w2i, identity, FP8, "w2")
            if n == N_A - 1:
                cc1A = _stats_payload_ar(nc, pools, stats1, 0, N_A, K1, "s1A")

        cc1B = _stats_payload_ar(nc, pools, stats1, N_A, BPC, K1, "s1B")
        mean1, rstd1 = _combine_stats(nc, pools, cc1A, cc1B, epst, "bn1")
        sc1, bi1 = _affine_vecs(nc, pools, g1, b1, mean1, rstd1, K1, K1, "1")

        # ---- phase B: act1 + conv2 per image ----
        stats2 = pools["stats"].tile([C, BPC * TPI * 6], F32, name="stats2")
        out2 = []
        cc2A = None
        for n in range(BPC):
            o1 = out1[n]
            # act1: relu(sc1*o1+bi1) on ACT; rint on DVE; min15 -> fp8 on DVE
            nc.scalar.activation(o1[:], o1[:],
                                 mybir.ActivationFunctionType.Relu,
                                 bias=bi1[:], scale=sc1[:])
            nc.vector.tensor_scalar(o1[:], o1[:], C23, C23,
                                    op0=mybir.AluOpType.add,
                                    op1=mybir.AluOpType.subtract)
            a1 = pools["a1"].tile([C, PW], FP8, name=f"a1_{n}", tag="a1")
            if n < 3:
                _zero_halo(nc, a1)
            a1r = a1.rearrange("p (h w) -> p h w", w=WP)
            nc.vector.tensor_scalar_min(
                a1r[:, 1:1 + H, 1:1 + W],
                o1.rearrange("p (h w) -> p h w", w=W), 15.0)
            o2 = pools["big"].tile([C, HW], F32R, name=f"o2_{n}", tag="bigbuf")
            _conv_image(nc, pools, w2T, a1r, o2, use_dr=USE_DR)
            for t in range(TPI):
                gi = n * TPI + t
                nc.vector.bn_stats(stats2[:, gi * 6:(gi + 1) * 6],
                                   o2[:, t * TN:(t + 1) * TN].bitcast(F32))
            out2.append(o2)
            if n == N_A - 1:
                cc2A = _stats_payload_ar(nc, pools, stats2, 0, N_A, K2, "s2A")

        cc2B = _stats_payload_ar(nc, pools, stats2, N_A, BPC, K2, "s2B")
        mean2, rstd2 = _combine_stats(nc, pools, cc2A, cc2B, epst, "bn2")
        sc2, bi2 = _affine_vecs(nc, pools, g2, b2, mean2, rstd2, K1, K2, "2")
        d1 = pools["stats"].tile([C, C], F32R, name="d1")
        nc.vector.tensor_scalar_mul(d1[:], identity[:], sc2[:])
        d2 = pools["stats"].tile([C, C], FP16, name="d2")
        nc.vector.tensor_scalar_mul(d2[:], identity[:], bi2[:])

        # ---- tail: PSUM = d1@o2 + i15@x16 + d2@ones ; rint/clip/scale ----
        d1r = d1[:]
        for n in range(BPC):
            o2 = out2[n]
            o2r = o2[:]
            xpr = xp16s[n].rearrange("p (h w) -> p h w", w=WP)
            for ci, (t0, ntil) in enumerate(CHUNKS):
                pool = pools["psA" if ci == 0 else "psB"]
                ps = pool.tile([C, PSTRIDE * ntil], F32, name=f"tl{ci}",
                               tag=f"cvch{ci}")
                for i in range(ntil):
                    t = t0 + i
                    sl = ps[:, i * PSTRIDE:i * PSTRIDE + TN]
                    nc.tensor.matmul(sl, d1r, o2r[:, t * TN:(t + 1) * TN],
                                     start=True, stop=False)
                    nc.tensor.matmul(sl, i15[:],
                                     xpr[:, RPT * t + 1:RPT * t + 1 + RPT,
                                         1:1 + W],
                                     start=False, stop=False)
                    nc.tensor.matmul(sl, d2[:], ones[:],
                                     start=False, stop=True)
                src = ps.rearrange("p (t c) -> p t c", c=PSTRIDE)[:, :, 0:TN]
                to = pools["stage"].tile([C, TN * ntil], F32, name=f"to{ci}",
                                         tag="stage")
                flat = to[:]
                dst = flat.rearrange("p (t c) -> p t c", c=TN)
                # rint via +2^23 (ACT computes in fp32; add rounds to integer)
                nc.scalar.activation(dst, src,
                                     mybir.ActivationFunctionType.Copy,
                                     bias=C23)
                nc.vector.tensor_scalar(flat, flat, C23, C23 + 15.0,
                                        op0=mybir.AluOpType.max,
                                        op1=mybir.AluOpType.min)
                nc.vector.tensor_scalar(flat, flat, C23, 1.0 / 15.0,
                                        op0=mybir.AluOpType.subtract,
                                        op1=mybir.AluOpType.mult)
                nc.sync.dma_start(
                    out_d[n][:, t0 * RPT:(t0 + ntil) * RPT, :],
                    flat.rearrange("p (h w) -> p h w", w=W))

    nc.compile()
    return nc


def _get_nc():
    if "nc" not in _CACHE:
        _CACHE["nc"] = build()
    return _CACHE["nc"]


def kernel(x, w1, w2, gamma1, beta1, gamma2, beta2, _trace=False):
    nc = _get_nc()
    x = np.ascontiguousarray(np.asarray(x, dtype=np.float32))
    in_common = {
        "w1": np.ascontiguousarray(np.asarray(w1, np.float32).reshape(C, C * 9)),
        "w2": np.ascontiguousarray(np.asarray(w2, np.float32).reshape(C, C * 9)),
        "gamma1": np.asarray(gamma1, np.float32).reshape(C, 1),
        "beta1": np.asarray(beta1, np.float32).reshape(C, 1),
        "gamma2": np.asarray(gamma2, np.float32).reshape(C, 1),
        "beta2": np.asarray(beta2, np.float32).reshape(C, 1),
    }
    in_maps = [dict(in_common, x=x[c * BPC:(c + 1) * BPC]) for c in range(N_CORES)]
    res = bass_utils.run_bass_kernel_spmd(nc, in_maps, core_ids=list(range(N_CORES)),
                                          trace=_trace)
    out = np.concatenate([res.results[c]["out"] for c in range(N_CORES)], axis=0)
    if _trace:
        _CACHE["last_exec_time_ns"] = res.exec_time_ns
        _CACHE["last_results"] = res
    return out


if __name__ == "__main__":
    nc = build()
    print("built ok")


# revision 39
# speedup vs baseline: 1.0561x; 1.0561x over previous
"""Trainium2 Bass kernel for quantized BasicBlock (DoReFa conv-bn-act x2 + residual).

Self-contained: builds an 8-core SPMD Bass kernel, shards the batch (64 -> 8x8),
runs via bass_utils.run_bass_kernel_spmd, gathers the full output.

Math (per core, batch shard of 8 images):
  W_int = 2*rint(tanh(w)*s + 7.5) - 15, s = 15/(2*max|tanh(w)|)   (odd ints, |.|<=15)
  conv1: S1 = conv3x3(fp16(x), W1_int)      == 15 * conv3x3(x, w_q1) + eps_fp16
  BN1 stats of S1 over (N,H,W) all-reduced across cores (split 6+2 images so the
  first AllReduce hides under the tail of conv1)
  act1  = min(rint(relu(S1*sc1 + bi1)), 15)  (ints 0..15, stored fp8e4m3)
  conv2: S2 = conv3x3(act1, W2_int)          == 225 * conv3x3(a_q, w_q2), exact
         (fp8 matmuls; dx-adjacent tap pairs fused via DoubleRow perf mode)
  BN2 stats of S2 all-reduced (same 6+2 split)
  tail : PSUM = diag(15*sc2)@S2 + (15*I)@x + diag(15*bi2)@ones   (f32r matmuls)
         out  = (clip(rint(PSUM), 0, 15)) / 15
"""
import sys
from contextlib import ExitStack

import numpy as np

for _p in ("/opt/trn_rl_repo",):
    if _p not in sys.path:
        sys.path.append(_p)

import concourse.bass as bass
import concourse.bass_isa as bass_isa
import concourse.bacc as bacc
import concourse.mybir as mybir
import concourse.tile as tile
from concourse import bass_utils
from concourse.bass import AP
from concourse.masks import make_identity

F32 = mybir.dt.float32
F32R = mybir.dt.float32r
FP16 = mybir.dt.float16
FP8 = mybir.dt.float8e4

N_CORES = 8
B, C, H, W = 64, 128, 56, 56
BPC = B // N_CORES            # images per core
HP, WP = H + 2, W + 2         # padded 58x58
PW = HP * WP                  # 3364
HW = H * W                    # 3136
RPT = 8                       # output rows per PSUM tile
TN = RPT * W                  # 448 columns per matmul
TPI = H // RPT                # 7 tiles per image
PSTRIDE = 512                 # PSUM bank stride in f32 elements
C23 = float(2 ** 23)
K1 = 15.0                     # conv1 PSUM = 15 * true conv
K2 = 225.0                    # conv2 PSUM = 225 * true conv
N_A = 4                       # images in the first (hidden) stats AllReduce
ROWS_A = 33                   # x rows feeding conv chunk A (+1 halo overlap)
USE_DR = True                 # DoubleRow fp8 pairing for conv2

TAPS = [(dy, dx) for dy in range(3) for dx in range(3)]

# conv PSUM chunks: (first tile, n tiles). 4+3 tiles -> 4+3 banks, 8th bank for
# the weight transposes.
CHUNKS = [(0, 4), (4, 3)]

_CACHE = {}


def _quant_weights(nc, pools, w_in, identity, ones_row, name):
    """DMA + DoReFa-quantize weights in-place on one (C, C*9) f32 tile.

    The cross-partition absmax runs on PE/DVE (transpose -> free-axis reduce
    -> matmul broadcast) instead of gpsimd: the Q7 custom-op launch costs
    ~15us on the critical path.
    """
    wp = pools["wprep"]
    trp = pools["psT"]
    wk = wp.tile([C, C * 9], F32, name=f"{name}_wk", tag="wk")
    half = C * 9 // 2
    nc.scalar.dma_start(wk[:, 0:half], w_in[:, 0:half])
    nc.scalar.dma_start(wk[:, half:], w_in[:, half:])
    am = wp.tile([C, 1], F32, name=f"{name}_am", tag="wam")
    nc.vector.tensor_reduce(am[:], wk[:], axis=mybir.AxisListType.X,
                            op=mybir.AluOpType.max, apply_absolute_value=True)
    nc.scalar.activation(wk[:], wk[:], mybir.ActivationFunctionType.Tanh)
    # partition max: transpose [C,1] -> [1,C], reduce on one lane, broadcast
    psr = trp.tile([C, C], F32, name=f"{name}_psr", tag="trps")
    nc.tensor.transpose(psr[0:1, 0:C], am[:], identity[:])
    amr = wp.tile([1, C], F32, name=f"{name}_amr", tag="wamr")
    nc.scalar.copy(amr[:], psr[0:1, 0:C])
    am0 = wp.tile([1, 1], F32, name=f"{name}_am0", tag="wam0")
    nc.vector.tensor_reduce(am0[:], amr[:], axis=mybir.AxisListType.X,
                            op=mybir.AluOpType.max)
    psb = trp.tile([C, C], F32, name=f"{name}_psb", tag="trps")
    nc.tensor.matmul(psb[0:C, 0:1], ones_row[:], am0[:], start=True, stop=True)
    amg = wp.tile([C, 1], F32, name=f"{name}_amg", tag="wamg")
    nc.scalar.copy(amg[:], psb[0:C, 0:1])
    s_t = wp.tile([C, 1], F32, name=f"{name}_s", tag="ws")
    nc.scalar.activation(s_t[:], amg[:], mybir.ActivationFunctionType.Tanh)
    nc.vector.reciprocal(s_t[:], s_t[:])
    nc.vector.tensor_scalar_mul(s_t[:], s_t[:], 7.5)
    # W_int = 2*rint(tanh*s + 7.5) - 15
    nc.vector.tensor_scalar(wk[:], wk[:], s_t[:], 7.5,
                            op0=mybir.AluOpType.mult, op1=mybir.AluOpType.add)
    nc.vector.tensor_scalar(wk[:], wk[:], C23, C23,
                            op0=mybir.AluOpType.add, op1=mybir.AluOpType.subtract)
    nc.vector.tensor_scalar(wk[:], wk[:], 2.0, 15.0,
                            op0=mybir.AluOpType.mult, op1=mybir.AluOpType.subtract)
    return wk


def _transpose_taps(nc, pools, wint, identity, out_dt, name):
    """Per-tap PE transpose of W_int (O,(I,t)) -> wT (I,(t,O)) in out_dt."""
    wp = pools["wconst"]
    trp = pools["psT"]
    wT = wp.tile([C, 9 * C], out_dt, name=f"{name}_T")
    wr = wint.rearrange("p (i t) -> p i t", t=9)
    for t in range(9):
        ps = trp.tile([C, C], F32, name=f"{name}_ps{t}", tag="trps")
        nc.tensor.transpose(ps[:], wr[:, :, t], identity[:])
        nc.scalar.copy(wT[:, t * C:(t + 1) * C], ps[:])
    return wT


def _warmup_allreduce_eps(nc, pools):
    """Tiny AllReduce at kernel start: warms up ncfw and produces the BN
    epsilon constant (8 * 1e-5/8) so it survives DCE."""
    sp = pools["stats"]
    dp = pools["dram"]
    eps8 = sp.tile([C, 1], F32, name="eps8")
    nc.gpsimd.memset(eps8[:], 1e-5 / N_CORES)
    cc_in = dp.tile([C, 1], F32, name="ccw_in")
    cc_out = dp.tile([C, 1], F32, name="ccw_out")
    nc.gpsimd.dma_start(cc_in[:], eps8[:])
    nc.gpsimd.collective_compute(
        "AllReduce", mybir.AluOpType.add,
        replica_groups=[list(range(N_CORES))],
        ins=[cc_in.opt()], outs=[cc_out.opt()],
    )
    epst = sp.tile([C, 1], F32, name="epst")
    nc.sync.dma_start(epst[:], cc_out[:])
    return epst


def _stats_payload_ar(nc, pools, stats, i0, i1, k_scale, name):
    """bn_aggr over images [i0, i1) -> payload (sum_x, sum_x2)/N_global in
    UNSCALED units -> AllReduce. Returns the DRAM tile holding the result."""
    sp = pools["stats"]
    dp = pools["dram"]
    nimg = i1 - i0
    wfrac = float(nimg) / float(B)
    loc = sp.tile([C, 2], F32, name=f"{name}_loc")
    nc.vector.bn_aggr(loc[:], stats[:, i0 * TPI * 6:i1 * TPI * 6].rearrange(
        "p (t k) -> p t k", k=3))
    mu = sp.tile([C, 1], F32, name=f"{name}_mu")
    nc.vector.tensor_scalar_mul(mu[:], loc[:, 0:1], 1.0 / k_scale)
    pay = sp.tile([C, 2], F32, name=f"{name}_pay")
    nc.vector.tensor_scalar_mul(pay[:, 0:1], mu[:], wfrac)
    # pay1 = wfrac * (var/k^2 + mu^2)
    msq = sp.tile([C, 1], F32, name=f"{name}_msq")
    nc.vector.scalar_tensor_tensor(msq[:], mu[:], wfrac, mu[:],
                                   op0=mybir.AluOpType.mult,
                                   op1=mybir.AluOpType.mult)
    nc.vector.scalar_tensor_tensor(pay[:, 1:2], loc[:, 1:2],
                                   wfrac / (k_scale * k_scale), msq[:],
                                   op0=mybir.AluOpType.mult,
                                   op1=mybir.AluOpType.add)
    cc_in = dp.tile([C, 2], F32, name=f"{name}_in")
    cc_out = dp.tile([C, 2], F32, name=f"{name}_out")
    nc.sync.dma_start(cc_in[:], pay[:])
    nc.gpsimd.collective_compute(
        "AllReduce", mybir.AluOpType.add,
        replica_groups=[list(range(N_CORES))],
        ins=[cc_in.opt()], outs=[cc_out.opt()],
    )
    return cc_out


def _combine_stats(nc, pools, ccA, ccB, epst, name):
    """Fetch both AllReduce results, combine -> (mean_u, rstd_u)."""
    sp = pools["stats"]
    gA = sp.tile([C, 2], F32, name=f"{name}_gA")
    gB = sp.tile([C, 2], F32, name=f"{name}_gB")
    nc.sync.dma_start(gA[:], ccA[:])
    nc.sync.dma_start(gB[:], ccB[:])
    gs = sp.tile([C, 2], F32, name=f"{name}_gs")
    nc.vector.tensor_tensor(gs[:], gA[:], gB[:], op=mybir.AluOpType.add)
    mean_g = gs[:, 0:1]
    m2 = sp.tile([C, 1], F32, name=f"{name}_m2")
    nc.vector.scalar_tensor_tensor(m2[:], mean_g, 1.0, mean_g,
                                   op0=mybir.AluOpType.mult,
                                   op1=mybir.AluOpType.mult)
    varg = sp.tile([C, 1], F32, name=f"{name}_var")
    nc.vector.scalar_tensor_tensor(varg[:], m2[:], -1.0, gs[:, 1:2],
                                   op0=mybir.AluOpType.mult,
                                   op1=mybir.AluOpType.add)
    std = sp.tile([C, 1], F32, name=f"{name}_std")
    nc.scalar.activation(std[:], varg[:], mybir.ActivationFunctionType.Sqrt,
                         bias=epst[:])
    rstd = sp.tile([C, 1], F32, name=f"{name}_rstd")
    nc.vector.reciprocal(rstd[:], std[:])
    return mean_g, rstd


def _affine_vecs(nc, pools, gamma, beta, mean_u, rstd_u, m_out, k_scale, name):
    """For y_out = m*bn(S/k): sc = m*gamma*rstd/k ; bi = m*(beta - mean_u*gamma*rstd)."""
    sp = pools["stats"]
    gr = sp.tile([C, 1], F32, name=f"gr{name}")
    nc.vector.scalar_tensor_tensor(gr[:], gamma[:], 1.0, rstd_u[:],
                                   op0=mybir.AluOpType.bypass,
                                   op1=mybir.AluOpType.mult)
    sc = sp.tile([C, 1], F32, name=f"sc{name}")
    nc.vector.tensor_scalar_mul(sc[:], gr[:], m_out / k_scale)
    negms = sp.tile([C, 1], F32, name=f"negms{name}")
    nc.vector.scalar_tensor_tensor(negms[:], mean_u, -1.0, gr[:],
                                   op0=mybir.AluOpType.mult,
                                   op1=mybir.AluOpType.mult)
    bi = sp.tile([C, 1], F32, name=f"bi{name}")
    nc.vector.scalar_tensor_tensor(bi[:], negms[:], 1.0, beta[:],
                                   op0=mybir.AluOpType.bypass,
                                   op1=mybir.AluOpType.add)
    nc.vector.tensor_scalar_mul(bi[:], bi[:], m_out)
    return sc, bi


def _dr_rhs(img_view, t, dy):
    """DoubleRow rhs: overlapping 4D AP [C, 2, RPT, W]; pair dim = dx 0/1
    (stride 1 fp8 element)."""
    base = img_view[:, RPT * t + dy: RPT * t + dy + RPT, 0:W]
    u = base.unsqueeze(1)
    ap = [list(p) for p in u.ap]
    ap[1] = [1, 2]
    return AP(u.tensor, u.offset, ap)


def _conv_image(nc, pools, wT, img_view, out_sb, use_dr, stats, n):
    """One image: 2 PSUM chunks; per tile accumulate 9 taps; per chunk a single
    strided ACT copy-out PSUM -> out_sb + per-tile bn_stats (emitted per chunk
    so the last image's stats reach the AllReduce payload sooner)."""
    for ci, (t0, ntil) in enumerate(CHUNKS):
        pool = pools["psA" if ci == 0 else "psB"]
        ps = pool.tile([C, PSTRIDE * ntil], F32, name=f"cv{ci}",
                       tag=f"cvch{ci}")
        for i in range(ntil):
            t = t0 + i
            sl = ps[:, i * PSTRIDE:i * PSTRIDE + TN]
            if use_dr:
                for dy in range(3):
                    lhsT = wT[:, (3 * dy) * C:(3 * dy + 2) * C].rearrange(
                        "p (two f) -> p two f", two=2)
                    nc.tensor.matmul(sl, lhsT, _dr_rhs(img_view, t, dy),
                                     start=(dy == 0), stop=False,
                                     perf_mode=mybir.MatmulPerfMode.DoubleRow)
                for dy in range(3):
                    k = 3 * dy + 2
                    rhs = img_view[:, RPT * t + dy: RPT * t + dy + RPT, 2:2 + W]
                    nc.tensor.matmul(sl, wT[:, k * C:(k + 1) * C], rhs,
                                     start=False, stop=(dy == 2))
            else:
                for k, (dy, dx) in enumerate(TAPS):
                    rhs = img_view[:, RPT * t + dy: RPT * t + dy + RPT,
                                   dx: dx + W]
                    nc.tensor.matmul(sl, wT[:, k * C:(k + 1) * C], rhs,
                                     start=(k == 0), stop=(k == 8))
        # single strided copy-out for the chunk
        src = ps.rearrange("p (t c) -> p t c", c=PSTRIDE)[:, :, 0:TN]
        dst = out_sb[:, t0 * TN:(t0 + ntil) * TN].rearrange(
            "p (t c) -> p t c", c=TN)
        nc.scalar.copy(dst, src)
        for i in range(ntil):
            t = t0 + i
            gi = n * TPI + t
            nc.vector.bn_stats(stats[:, gi * 6:(gi + 1) * 6],
                               out_sb[:, t * TN:(t + 1) * TN].bitcast(F32))


def _zero_halo(nc, xb, dt_zero=0.0):
    """Zero the 1-px halo of a padded [C, PW] image tile (3 memsets)."""
    xbr = xb.rearrange("p (h w) -> p h w", w=WP)
    nc.gpsimd.memset(xbr[:, 0, :], dt_zero)
    nc.gpsimd.memset(xbr[:, HP - 1, :], dt_zero)
    side = xb[:, WP - 1:WP - 1 + (HP - 1) * WP].rearrange(
        "p (a b) -> p a b", b=WP)
    nc.gpsimd.memset(side[:, :, 0:2], dt_zero)


def build():
    nc = bacc.Bacc("TRN2", target_bir_lowering=False, debug=False,
                   enable_asserts=False, num_devices=N_CORES)
    x_in = nc.dram_tensor("x", [BPC, C, H, W], F32, kind="ExternalInput").ap()
    w1_in = nc.dram_tensor("w1", [C, C * 9], F32, kind="ExternalInput").ap()
    w2_in = nc.dram_tensor("w2", [C, C * 9], F32, kind="ExternalInput").ap()
    g1_in = nc.dram_tensor("gamma1", [C, 1], F32, kind="ExternalInput").ap()
    b1_in = nc.dram_tensor("beta1", [C, 1], F32, kind="ExternalInput").ap()
    g2_in = nc.dram_tensor("gamma2", [C, 1], F32, kind="ExternalInput").ap()
    b2_in = nc.dram_tensor("beta2", [C, 1], F32, kind="ExternalInput").ap()
    out_d = nc.dram_tensor("out", [BPC, C, H, W], F32, kind="ExternalOutput").ap()

    with tile.TileContext(nc) as tc, ExitStack() as ctx:
        pools = {
            "wprep": ctx.enter_context(tc.tile_pool(name="wprep", bufs=1)),
            "wconst": ctx.enter_context(tc.tile_pool(name="wconst", bufs=1)),
            "stats": ctx.enter_context(tc.tile_pool(name="stats", bufs=1)),
            "xp16": ctx.enter_context(tc.tile_pool(name="xp16", bufs=8)),
            "big": ctx.enter_context(tc.tile_pool(name="big", bufs=8)),
            "a1": ctx.enter_context(tc.tile_pool(name="a1", bufs=2)),
            # shared staging ring: weight-quant scratch, x fp32 staging, and
            # tail result buffers all rotate through 3 slots
            "stage": ctx.enter_context(tc.tile_pool(name="stage", bufs=3)),
            "psA": ctx.enter_context(
                tc.tile_pool(name="psA", bufs=1, space="PSUM")),
            "psB": ctx.enter_context(
                tc.tile_pool(name="psB", bufs=1, space="PSUM")),
            "psT": ctx.enter_context(
                tc.tile_pool(name="psT", bufs=1, space="PSUM")),
            "dram": ctx.enter_context(tc.tile_pool(name="dram", bufs=12,
                                                   space="DRAM")),
        }
        consts = pools["wconst"]

        # per-channel params (sync queue; scalar queue is loading w1)
        g1 = consts.tile([C, 1], F32, name="g1"); nc.sync.dma_start(g1[:], g1_in[:])
        b1 = consts.tile([C, 1], F32, name="b1"); nc.sync.dma_start(b1[:], b1_in[:])
        g2 = consts.tile([C, 1], F32, name="g2"); nc.sync.dma_start(g2[:], g2_in[:])
        b2 = consts.tile([C, 1], F32, name="b2"); nc.sync.dma_start(b2[:], b2_in[:])

        identity = consts.tile([C, C], F32, name="identity")
        make_identity(nc, identity[:])
        # fp16 identity*15 and ones for the tail matmuls (fp16 weights keep
        # FWL weight loads fast); the diag(sc2) stays f32r for precision
        i15 = consts.tile([C, C], FP16, name="i15")
        nc.vector.tensor_scalar_mul(i15[:], identity[:], 15.0)
        ones = consts.tile([C, TN], FP16, name="ones")
        nc.vector.memset(ones[:], 1.0)
        ones_row = consts.tile([1, C], F32, name="ones_row")
        nc.vector.memset(ones_row[:], 1.0)

        epst = _warmup_allreduce_eps(nc, pools)

        # ---- w1 quant + transpose (critical path to first conv MM) ----
        w1i = _quant_weights(nc, pools, w1_in, identity, ones_row, "w1")
        w1T = _transpose_taps(nc, pools, w1i, identity, FP16, "w1")

        # ---- phase A: conv1 per image (single fp16 pass) ----
        stats1 = pools["stats"].tile([C, BPC * TPI * 6], F32, name="stats1")
        out1 = []
        cc1A = None
        w2T = None
        xp16s = []
        for n in range(BPC):
            # x staged fp32 (sync DMA, two halves) then ACT-converted into the
            # padded fp16 image; conv chunk A only depends on the first 33
            # rows (region-granular deps). The fp16 copy also serves as the
            # tail's residual (no reload).
            xin = x_in[n].rearrange("c h w -> c (h w)")
            xp = pools["xp16"].tile([C, PW], FP16, name=f"xp{n}", tag="xp")
            _zero_halo(nc, xp)
            xpr = xp.rearrange("p (h w) -> p h w", w=WP)
            xsA = pools["stage"].tile([C, ROWS_A * W], F32, name=f"xsA{n}",
                                      tag="stage")
            nc.sync.dma_start(xsA[:], xin[:, 0:ROWS_A * W])
            nc.scalar.activation(xpr[:, 1:1 + ROWS_A, 1:1 + W],
                                 xsA.rearrange("p (h w) -> p h w", w=W),
                                 mybir.ActivationFunctionType.Copy)
            xsB = pools["stage"].tile([C, (H - ROWS_A) * W], F32,
                                      name=f"xsB{n}", tag="stage")
            nc.sync.dma_start(xsB[:], xin[:, ROWS_A * W:])
            nc.scalar.activation(xpr[:, 1 + ROWS_A:1 + H, 1:1 + W],
                                 xsB.rearrange("p (h w) -> p h w", w=W),
                                 mybir.ActivationFunctionType.Copy)
            xp16s.append(xp)
            o1 = pools["big"].tile([C, HW], F32, name=f"o1_{n}", tag="bigbuf")
            _conv_image(nc, pools, w1T, xpr, o1, False, stats1, n)
            out1.append(o1)
            if n == 0:
                # w2 prep hides under conv1 of images 1..7
                w2i = _quant_weights(nc, pools, w2_in, identity, ones_row, "w2")
                w2T = _transpose_taps(nc, pools, w2i, identity, FP8, "w2")
            if n == N_A - 1:
                cc1A = _stats_payload_ar(nc, pools, stats1, 0, N_A, K1, "s1A")

        cc1B = _stats_payload_ar(nc, pools, stats1, N_A, BPC, K1, "s1B")
        mean1, rstd1 = _combine_stats(nc, pools, cc1A, cc1B, epst, "bn1")
        sc1, bi1 = _affine_vecs(nc, pools, g1, b1, mean1, rstd1, K1, K1, "1")

        # ---- phase B: act1 + conv2 per image ----
        stats2 = pools["stats"].tile([C, BPC * TPI * 6], F32, name="stats2")
        out2 = []
        cc2A = None
        for n in range(BPC):
            o1 = out1[n]
            a1 = pools["a1"].tile([C, PW], FP8, name=f"a1_{n}", tag="a1")
            if n < 2:
                _zero_halo(nc, a1)
            a1r = a1.rearrange("p (h w) -> p h w", w=WP)
            o1r = o1.rearrange("p (h w) -> p h w", w=W)
            # act1: relu(sc1*o1+bi1) on ACT; rint on DVE; min15 -> fp8 on DVE
            # (image 0 in two row-segments so conv2's first chunk starts early)
            segs = [(0, ROWS_A), (ROWS_A, H)] if n == 0 else [(0, H)]
            for r0, r1 in segs:
                seg = o1[:, r0 * W:r1 * W]
                nc.scalar.activation(seg, seg,
                                     mybir.ActivationFunctionType.Relu,
                                     bias=bi1[:], scale=sc1[:])
                nc.vector.tensor_scalar(seg, seg, C23, C23,
                                        op0=mybir.AluOpType.add,
                                        op1=mybir.AluOpType.subtract)
                nc.vector.tensor_scalar_min(
                    a1r[:, 1 + r0:1 + r1, 1:1 + W], o1r[:, r0:r1, :], 15.0)
            o2 = pools["big"].tile([C, HW], F32R, name=f"o2_{n}", tag="bigbuf")
            _conv_image(nc, pools, w2T, a1r, o2, USE_DR, stats2, n)
            out2.append(o2)
            if n == N_A - 1:
                cc2A = _stats_payload_ar(nc, pools, stats2, 0, N_A, K2, "s2A")

        cc2B = _stats_payload_ar(nc, pools, stats2, N_A, BPC, K2, "s2B")
        mean2, rstd2 = _combine_stats(nc, pools, cc2A, cc2B, epst, "bn2")
        sc2, bi2 = _affine_vecs(nc, pools, g2, b2, mean2, rstd2, K1, K2, "2")
        d1 = pools["stats"].tile([C, C], F32R, name="d1")
        nc.vector.tensor_scalar_mul(d1[:], identity[:], sc2[:])
        d2 = pools["stats"].tile([C, C], FP16, name="d2")
        nc.vector.tensor_scalar_mul(d2[:], identity[:], bi2[:])

        # ---- tail: PSUM = d1@o2 + i15@x16 + d2@ones ; rint/clip/scale ----
        d1r = d1[:]
        for n in range(BPC):
            o2 = out2[n]
            o2r = o2[:]
            xpr = xp16s[n].rearrange("p (h w) -> p h w", w=WP)
            for ci, (t0, ntil) in enumerate(CHUNKS):
                pool = pools["psA" if ci == 0 else "psB"]
                ps = pool.tile([C, PSTRIDE * ntil], F32, name=f"tl{ci}",
                               tag=f"cvch{ci}")
                for i in range(ntil):
                    t = t0 + i
                    sl = ps[:, i * PSTRIDE:i * PSTRIDE + TN]
                    nc.tensor.matmul(sl, d1r, o2r[:, t * TN:(t + 1) * TN],
                                     start=True, stop=False)
                    nc.tensor.matmul(sl, i15[:],
                                     xpr[:, RPT * t + 1:RPT * t + 1 + RPT,
                                         1:1 + W],
                                     start=False, stop=False)
                    nc.tensor.matmul(sl, d2[:], ones[:],
                                     start=False, stop=True)
                src = ps.rearrange("p (t c) -> p t c", c=PSTRIDE)[:, :, 0:TN]
                to = pools["stage"].tile([C, TN * ntil], F32, name=f"to{ci}",
                                         tag="stage")
                flat = to[:]
                dst = flat.rearrange("p (t c) -> p t c", c=TN)
                # rint via +2^23 (ACT computes in fp32; add rounds to integer)
                nc.scalar.activation(dst, src,
                                     mybir.ActivationFunctionType.Copy,
                                     bias=C23)
                nc.vector.tensor_scalar(flat, flat, C23, C23 + 15.0,
                                        op0=mybir.AluOpType.max,
                                        op1=mybir.AluOpType.min)
                nc.vector.tensor_scalar(flat, flat, C23, 1.0 / 15.0,
                                        op0=mybir.AluOpType.subtract,
                                        op1=mybir.AluOpType.mult)
                nc.sync.dma_start(
                    out_d[n][:, t0 * RPT:(t0 + ntil) * RPT, :],
                    flat.rearrange("p (h w) -> p h w", w=W))

    nc.compile()
    return nc


def _get_nc():
    if "nc" not in _CACHE:
        _CACHE["nc"] = build()
    return _CACHE["nc"]


def kernel(x, w1, w2, gamma1, beta1, gamma2, beta2, _trace=False):
    nc = _get_nc()
    x = np.ascontiguousarray(np.asarray(x, dtype=np.float32))
    in_common = {
        "w1": np.ascontiguousarray(np.asarray(w1, np.float32).reshape(C, C * 9)),
        "w2": np.ascontiguousarray(np.asarray(w2, np.float32).reshape(C, C * 9)),
        "gamma1": np.asarray(gamma1, np.float32).reshape(C, 1),
        "beta1": np.asarray(beta1, np.float32).reshape(C, 1),
        "gamma2": np.asarray(gamma2, np.float32).reshape(C, 1),
        "beta2": np.asarray(beta2, np.float32).reshape(C, 1),
    }
    in_maps = [dict(in_common, x=x[c * BPC:(c + 1) * BPC]) for c in range(N_CORES)]
    res = bass_utils.run_bass_kernel_spmd(nc, in_maps, core_ids=list(range(N_CORES)),
                                          trace=_trace)
    out = np.concatenate([res.results[c]["out"] for c in range(N_CORES)], axis=0)
    if _trace:
        _CACHE["last_exec_time_ns"] = res.exec_time_ns
        _CACHE["last_results"] = res
    return out


if __name__ == "__main__":
    nc = build()
    print("built ok")


# revision 41
# speedup vs baseline: 1.0593x; 1.0030x over previous
"""Trainium2 Bass kernel for quantized BasicBlock (DoReFa conv-bn-act x2 + residual).

Self-contained: builds an 8-core SPMD Bass kernel, shards the batch (64 -> 8x8),
runs via bass_utils.run_bass_kernel_spmd, gathers the full output.

Math (per core, batch shard of 8 images):
  W_int = 2*rint(tanh(w)*s + 7.5) - 15, s = 15/(2*max|tanh(w)|)   (odd ints, |.|<=15)
  conv1: S1 = conv3x3(fp16(x), W1_int)      == 15 * conv3x3(x, w_q1) + eps_fp16
  BN1 stats of S1 over (N,H,W) all-reduced across cores (split 6+2 images so the
  first AllReduce hides under the tail of conv1)
  act1  = min(rint(relu(S1*sc1 + bi1)), 15)  (ints 0..15, stored fp8e4m3)
  conv2: S2 = conv3x3(act1, W2_int)          == 225 * conv3x3(a_q, w_q2), exact
         (fp8 matmuls; dx-adjacent tap pairs fused via DoubleRow perf mode)
  BN2 stats of S2 all-reduced (same 6+2 split)
  tail : PSUM = diag(15*sc2)@S2 + (15*I)@x + diag(15*bi2)@ones   (f32r matmuls)
         out  = (clip(rint(PSUM), 0, 15)) / 15
"""
import sys
from contextlib import ExitStack

import numpy as np

for _p in ("/opt/trn_rl_repo",):
    if _p not in sys.path:
        sys.path.append(_p)

import concourse.bass as bass
import concourse.bass_isa as bass_isa
import concourse.bacc as bacc
import concourse.mybir as mybir
import concourse.tile as tile
from concourse import bass_utils
from concourse.bass import AP
from concourse.masks import make_identity

F32 = mybir.dt.float32
F32R = mybir.dt.float32r
FP16 = mybir.dt.float16
FP8 = mybir.dt.float8e4

N_CORES = 8
B, C, H, W = 64, 128, 56, 56
BPC = B // N_CORES            # images per core
HP, WP = H + 2, W + 2         # padded 58x58
PW = HP * WP                  # 3364
HW = H * W                    # 3136
RPT = 8                       # output rows per PSUM tile
TN = RPT * W                  # 448 columns per matmul
TPI = H // RPT                # 7 tiles per image
PSTRIDE = 512                 # PSUM bank stride in f32 elements
C23 = float(2 ** 23)
K1 = 15.0                     # conv1 PSUM = 15 * true conv
K2 = 225.0                    # conv2 PSUM = 225 * true conv
N_A = 4                       # images in the first (hidden) stats AllReduce
ROWS_A = 33                   # x rows feeding conv chunk A (+1 halo overlap)
USE_DR = True                 # DoubleRow fp8 pairing for conv2

TAPS = [(dy, dx) for dy in range(3) for dx in range(3)]

# conv PSUM chunks: (first tile, n tiles). 4+3 tiles -> 4+3 banks, 8th bank for
# the weight transposes.
CHUNKS = [(0, 4), (4, 3)]

_CACHE = {}


def _quant_weights(nc, pools, w_in, identity, ones_row, name):
    """DMA + DoReFa-quantize weights in-place on one (C, C*9) f32 tile.

    The cross-partition absmax runs on PE/DVE (transpose -> free-axis reduce
    -> matmul broadcast) instead of gpsimd: the Q7 custom-op launch costs
    ~15us on the critical path.
    """
    wp = pools["wprep"]
    trp = pools["psT"]
    wk = wp.tile([C, C * 9], F32, name=f"{name}_wk", tag="wk")
    half = C * 9 // 2
    nc.scalar.dma_start(wk[:, 0:half], w_in[:, 0:half])
    nc.scalar.dma_start(wk[:, half:], w_in[:, half:])
    am = wp.tile([C, 1], F32, name=f"{name}_am", tag="wam")
    nc.vector.tensor_reduce(am[:], wk[:], axis=mybir.AxisListType.X,
                            op=mybir.AluOpType.max, apply_absolute_value=True)
    nc.scalar.activation(wk[:], wk[:], mybir.ActivationFunctionType.Tanh)
    # partition max: transpose [C,1] -> [1,C], reduce on one lane, broadcast
    psr = trp.tile([C, C], F32, name=f"{name}_psr", tag="trps")
    nc.tensor.transpose(psr[0:1, 0:C], am[:], identity[:])
    amr = wp.tile([1, C], F32, name=f"{name}_amr", tag="wamr")
    nc.scalar.copy(amr[:], psr[0:1, 0:C])
    am0 = wp.tile([1, 1], F32, name=f"{name}_am0", tag="wam0")
    nc.vector.tensor_reduce(am0[:], amr[:], axis=mybir.AxisListType.X,
                            op=mybir.AluOpType.max)
    psb = trp.tile([C, C], F32, name=f"{name}_psb", tag="trps")
    nc.tensor.matmul(psb[0:C, 0:1], ones_row[:], am0[:], start=True, stop=True)
    amg = wp.tile([C, 1], F32, name=f"{name}_amg", tag="wamg")
    nc.scalar.copy(amg[:], psb[0:C, 0:1])
    s_t = wp.tile([C, 1], F32, name=f"{name}_s", tag="ws")
    nc.scalar.activation(s_t[:], amg[:], mybir.ActivationFunctionType.Tanh)
    nc.vector.reciprocal(s_t[:], s_t[:])
    nc.vector.tensor_scalar_mul(s_t[:], s_t[:], 7.5)
    # W_int = 2*rint(tanh*s + 7.5) - 15
    nc.vector.tensor_scalar(wk[:], wk[:], s_t[:], 7.5,
                            op0=mybir.AluOpType.mult, op1=mybir.AluOpType.add)
    nc.vector.tensor_scalar(wk[:], wk[:], C23, C23,
                            op0=mybir.AluOpType.add, op1=mybir.AluOpType.subtract)
    nc.vector.tensor_scalar(wk[:], wk[:], 2.0, 15.0,
                            op0=mybir.AluOpType.mult, op1=mybir.AluOpType.subtract)
    return wk


def _transpose_taps(nc, pools, wint, identity, out_dt, name):
    """Per-tap PE transpose of W_int (O,(I,t)) -> wT (I,(t,O)) in out_dt."""
    wp = pools["wconst"]
    trp = pools["psT"]
    wT = wp.tile([C, 9 * C], out_dt, name=f"{name}_T")
    wr = wint.rearrange("p (i t) -> p i t", t=9)
    for t in range(9):
        ps = trp.tile([C, C], F32, name=f"{name}_ps{t}", tag="trps")
        nc.tensor.transpose(ps[:], wr[:, :, t], identity[:])
        nc.scalar.copy(wT[:, t * C:(t + 1) * C], ps[:])
    return wT


def _warmup_allreduce_eps(nc, pools):
    """Tiny AllReduce at kernel start: warms up ncfw and produces the BN
    epsilon constant (8 * 1e-5/8) so it survives DCE."""
    sp = pools["stats"]
    dp = pools["dram"]
    eps8 = sp.tile([C, 1], F32, name="eps8")
    nc.gpsimd.memset(eps8[:], 1e-5 / N_CORES)
    cc_in = dp.tile([C, 1], F32, name="ccw_in")
    cc_out = dp.tile([C, 1], F32, name="ccw_out")
    nc.gpsimd.dma_start(cc_in[:], eps8[:])
    nc.gpsimd.collective_compute(
        "AllReduce", mybir.AluOpType.add,
        replica_groups=[list(range(N_CORES))],
        ins=[cc_in.opt()], outs=[cc_out.opt()],
    )
    epst = sp.tile([C, 1], F32, name="epst")
    nc.sync.dma_start(epst[:], cc_out[:])
    return epst


def _stats_payload_ar(nc, pools, stats, i0, i1, k_scale, name):
    """bn_aggr over images [i0, i1) -> payload (sum_x, sum_x2)/N_global in
    UNSCALED units -> AllReduce. Returns the DRAM tile holding the result."""
    sp = pools["stats"]
    dp = pools["dram"]
    nimg = i1 - i0
    wfrac = float(nimg) / float(B)
    loc = sp.tile([C, 2], F32, name=f"{name}_loc")
    nc.vector.bn_aggr(loc[:], stats[:, i0 * TPI * 6:i1 * TPI * 6].rearrange(
        "p (t k) -> p t k", k=3))
    mu = sp.tile([C, 1], F32, name=f"{name}_mu")
    nc.vector.tensor_scalar_mul(mu[:], loc[:, 0:1], 1.0 / k_scale)
    pay = sp.tile([C, 2], F32, name=f"{name}_pay")
    nc.vector.tensor_scalar_mul(pay[:, 0:1], mu[:], wfrac)
    # pay1 = wfrac * (var/k^2 + mu^2)
    msq = sp.tile([C, 1], F32, name=f"{name}_msq")
    nc.vector.scalar_tensor_tensor(msq[:], mu[:], wfrac, mu[:],
                                   op0=mybir.AluOpType.mult,
                                   op1=mybir.AluOpType.mult)
    nc.vector.scalar_tensor_tensor(pay[:, 1:2], loc[:, 1:2],
                                   wfrac / (k_scale * k_scale), msq[:],
                                   op0=mybir.AluOpType.mult,
                                   op1=mybir.AluOpType.add)
    cc_in = dp.tile([C, 2], F32, name=f"{name}_in")
    cc_out = dp.tile([C, 2], F32, name=f"{name}_out")
    nc.sync.dma_start(cc_in[:], pay[:])
    nc.gpsimd.collective_compute(
        "AllReduce", mybir.AluOpType.add,
        replica_groups=[list(range(N_CORES))],
        ins=[cc_in.opt()], outs=[cc_out.opt()],
    )
    return cc_out


def _combine_stats(nc, pools, ccA, ccB, epst, name):
    """Fetch both AllReduce results, combine -> (mean_u, rstd_u)."""
    sp = pools["stats"]
    gA = sp.tile([C, 2], F32, name=f"{name}_gA")
    gB = sp.tile([C, 2], F32, name=f"{name}_gB")
    nc.sync.dma_start(gA[:], ccA[:])
    nc.sync.dma_start(gB[:], ccB[:])
    gs = sp.tile([C, 2], F32, name=f"{name}_gs")
    nc.vector.tensor_tensor(gs[:], gA[:], gB[:], op=mybir.AluOpType.add)
    mean_g = gs[:, 0:1]
    m2 = sp.tile([C, 1], F32, name=f"{name}_m2")
    nc.vector.scalar_tensor_tensor(m2[:], mean_g, 1.0, mean_g,
                                   op0=mybir.AluOpType.mult,
                                   op1=mybir.AluOpType.mult)
    varg = sp.tile([C, 1], F32, name=f"{name}_var")
    nc.vector.scalar_tensor_tensor(varg[:], m2[:], -1.0, gs[:, 1:2],
                                   op0=mybir.AluOpType.mult,
                                   op1=mybir.AluOpType.add)
    std = sp.tile([C, 1], F32, name=f"{name}_std")
    nc.scalar.activation(std[:], varg[:], mybir.ActivationFunctionType.Sqrt,
                         bias=epst[:])
    rstd = sp.tile([C, 1], F32, name=f"{name}_rstd")
    nc.vector.reciprocal(rstd[:], std[:])
    return mean_g, rstd


def _affine_vecs(nc, pools, gamma, beta, mean_u, rstd_u, m_out, k_scale, name):
    """For y_out = m*bn(S/k): sc = m*gamma*rstd/k ; bi = m*(beta - mean_u*gamma*rstd)."""
    sp = pools["stats"]
    gr = sp.tile([C, 1], F32, name=f"gr{name}")
    nc.vector.scalar_tensor_tensor(gr[:], gamma[:], 1.0, rstd_u[:],
                                   op0=mybir.AluOpType.bypass,
                                   op1=mybir.AluOpType.mult)
    sc = sp.tile([C, 1], F32, name=f"sc{name}")
    nc.vector.tensor_scalar_mul(sc[:], gr[:], m_out / k_scale)
    negms = sp.tile([C, 1], F32, name=f"negms{name}")
    nc.vector.scalar_tensor_tensor(negms[:], mean_u, -1.0, gr[:],
                                   op0=mybir.AluOpType.mult,
                                   op1=mybir.AluOpType.mult)
    bi = sp.tile([C, 1], F32, name=f"bi{name}")
    nc.vector.scalar_tensor_tensor(bi[:], negms[:], 1.0, beta[:],
                                   op0=mybir.AluOpType.bypass,
                                   op1=mybir.AluOpType.add)
    nc.vector.tensor_scalar_mul(bi[:], bi[:], m_out)
    return sc, bi


def _dr_rhs(img_view, t, dy):
    """DoubleRow rhs: overlapping 4D AP [C, 2, RPT, W]; pair dim = dx 0/1
    (stride 1 fp8 element)."""
    base = img_view[:, RPT * t + dy: RPT * t + dy + RPT, 0:W]
    u = base.unsqueeze(1)
    ap = [list(p) for p in u.ap]
    ap[1] = [1, 2]
    return AP(u.tensor, u.offset, ap)


def _conv_image(nc, pools, wT, img_view, out_sb, use_dr, stats, n):
    """One image: 2 PSUM chunks; per tile accumulate 9 taps; per chunk a single
    strided ACT copy-out PSUM -> out_sb + per-tile bn_stats (emitted per chunk
    so the last image's stats reach the AllReduce payload sooner)."""
    for ci, (t0, ntil) in enumerate(CHUNKS):
        pool = pools["psA" if ci == 0 else "psB"]
        ps = pool.tile([C, PSTRIDE * ntil], F32, name=f"cv{ci}",
                       tag=f"cvch{ci}")
        for i in range(ntil):
            t = t0 + i
            sl = ps[:, i * PSTRIDE:i * PSTRIDE + TN]
            if use_dr:
                for dy in range(3):
                    lhsT = wT[:, (3 * dy) * C:(3 * dy + 2) * C].rearrange(
                        "p (two f) -> p two f", two=2)
                    nc.tensor.matmul(sl, lhsT, _dr_rhs(img_view, t, dy),
                                     start=(dy == 0), stop=False,
                                     perf_mode=mybir.MatmulPerfMode.DoubleRow)
                for dy in range(3):
                    k = 3 * dy + 2
                    rhs = img_view[:, RPT * t + dy: RPT * t + dy + RPT, 2:2 + W]
                    nc.tensor.matmul(sl, wT[:, k * C:(k + 1) * C], rhs,
                                     start=False, stop=(dy == 2))
            else:
                for k, (dy, dx) in enumerate(TAPS):
                    rhs = img_view[:, RPT * t + dy: RPT * t + dy + RPT,
                                   dx: dx + W]
                    nc.tensor.matmul(sl, wT[:, k * C:(k + 1) * C], rhs,
                                     start=(k == 0), stop=(k == 8))
        # single strided copy-out for the chunk
        src = ps.rearrange("p (t c) -> p t c", c=PSTRIDE)[:, :, 0:TN]
        dst = out_sb[:, t0 * TN:(t0 + ntil) * TN].rearrange(
            "p (t c) -> p t c", c=TN)
        nc.scalar.copy(dst, src)
        for i in range(ntil):
            t = t0 + i
            gi = n * TPI + t
            nc.vector.bn_stats(stats[:, gi * 6:(gi + 1) * 6],
                               out_sb[:, t * TN:(t + 1) * TN].bitcast(F32))


def _zero_halo(nc, xb, dt_zero=0.0):
    """Zero the 1-px halo of a padded [C, PW] image tile (3 memsets)."""
    xbr = xb.rearrange("p (h w) -> p h w", w=WP)
    nc.gpsimd.memset(xbr[:, 0, :], dt_zero)
    nc.gpsimd.memset(xbr[:, HP - 1, :], dt_zero)
    side = xb[:, WP - 1:WP - 1 + (HP - 1) * WP].rearrange(
        "p (a b) -> p a b", b=WP)
    nc.gpsimd.memset(side[:, :, 0:2], dt_zero)


def build():
    nc = bacc.Bacc("TRN2", target_bir_lowering=False, debug=False,
                   enable_asserts=False, num_devices=N_CORES)
    x_in = nc.dram_tensor("x", [BPC, C, H, W], F32, kind="ExternalInput").ap()
    w1_in = nc.dram_tensor("w1", [C, C * 9], F32, kind="ExternalInput").ap()
    w2_in = nc.dram_tensor("w2", [C, C * 9], F32, kind="ExternalInput").ap()
    g1_in = nc.dram_tensor("gamma1", [C, 1], F32, kind="ExternalInput").ap()
    b1_in = nc.dram_tensor("beta1", [C, 1], F32, kind="ExternalInput").ap()
    g2_in = nc.dram_tensor("gamma2", [C, 1], F32, kind="ExternalInput").ap()
    b2_in = nc.dram_tensor("beta2", [C, 1], F32, kind="ExternalInput").ap()
    out_d = nc.dram_tensor("out", [BPC, C, H, W], F32, kind="ExternalOutput").ap()

    with tile.TileContext(nc) as tc, ExitStack() as ctx:
        pools = {
            "wprep": ctx.enter_context(tc.tile_pool(name="wprep", bufs=1)),
            "wconst": ctx.enter_context(tc.tile_pool(name="wconst", bufs=1)),
            "stats": ctx.enter_context(tc.tile_pool(name="stats", bufs=1)),
            "xp16": ctx.enter_context(tc.tile_pool(name="xp16", bufs=8)),
            "big": ctx.enter_context(tc.tile_pool(name="big", bufs=8)),
            "a1": ctx.enter_context(tc.tile_pool(name="a1", bufs=2)),
            # shared staging ring: weight-quant scratch, x fp32 staging, and
            # tail result buffers all rotate through 3 slots
            "stage": ctx.enter_context(tc.tile_pool(name="stage", bufs=3)),
            "psA": ctx.enter_context(
                tc.tile_pool(name="psA", bufs=1, space="PSUM")),
            "psB": ctx.enter_context(
                tc.tile_pool(name="psB", bufs=1, space="PSUM")),
            "psT": ctx.enter_context(
                tc.tile_pool(name="psT", bufs=1, space="PSUM")),
            "dram": ctx.enter_context(tc.tile_pool(name="dram", bufs=12,
                                                   space="DRAM")),
        }
        consts = pools["wconst"]

        # per-channel params (sync queue; scalar queue is loading w1)
        g1 = consts.tile([C, 1], F32, name="g1"); nc.sync.dma_start(g1[:], g1_in[:])
        b1 = consts.tile([C, 1], F32, name="b1"); nc.sync.dma_start(b1[:], b1_in[:])
        g2 = consts.tile([C, 1], F32, name="g2"); nc.sync.dma_start(g2[:], g2_in[:])
        b2 = consts.tile([C, 1], F32, name="b2"); nc.sync.dma_start(b2[:], b2_in[:])

        identity = consts.tile([C, C], F32, name="identity")
        make_identity(nc, identity[:])
        # fp16 identity*15 and ones for the tail matmuls (fp16 weights keep
        # FWL weight loads fast); the diag(sc2) stays f32r for precision
        i15 = consts.tile([C, C], FP16, name="i15")
        nc.vector.tensor_scalar_mul(i15[:], identity[:], 15.0)
        ones = consts.tile([C, TN], FP16, name="ones")
        nc.vector.memset(ones[:], 1.0)
        ones_row = consts.tile([1, C], F32, name="ones_row")
        nc.vector.memset(ones_row[:], 1.0)

        epst = _warmup_allreduce_eps(nc, pools)

        # ---- w1 quant + transpose (critical path to first conv MM) ----
        w1i = _quant_weights(nc, pools, w1_in, identity, ones_row, "w1")
        w1T = _transpose_taps(nc, pools, w1i, identity, FP16, "w1")

        # ---- phase A: conv1 per image (single fp16 pass) ----
        stats1 = pools["stats"].tile([C, BPC * TPI * 6], F32, name="stats1")
        out1 = []
        cc1A = None
        w2T = None
        xp16s = []
        for n in range(BPC):
            # x staged fp32 (sync DMA, two halves) then ACT-converted into the
            # padded fp16 image; conv chunk A only depends on the first 33
            # rows (region-granular deps). The fp16 copy also serves as the
            # tail's residual (no reload).
            xin = x_in[n].rearrange("c h w -> c (h w)")
            xp = pools["xp16"].tile([C, PW], FP16, name=f"xp{n}", tag="xp")
            _zero_halo(nc, xp)
            xpr = xp.rearrange("p (h w) -> p h w", w=WP)
            # converts run on DVE so they never queue behind the ACT copyouts
            xsA = pools["stage"].tile([C, ROWS_A * W], F32, name=f"xsA{n}",
                                      tag="stage")
            nc.sync.dma_start(xsA[:], xin[:, 0:ROWS_A * W])
            nc.vector.tensor_copy(xpr[:, 1:1 + ROWS_A, 1:1 + W],
                                  xsA.rearrange("p (h w) -> p h w", w=W))
            xsB = pools["stage"].tile([C, (H - ROWS_A) * W], F32,
                                      name=f"xsB{n}", tag="stage")
            nc.sync.dma_start(xsB[:], xin[:, ROWS_A * W:])
            nc.vector.tensor_copy(xpr[:, 1 + ROWS_A:1 + H, 1:1 + W],
                                  xsB.rearrange("p (h w) -> p h w", w=W))
            xp16s.append(xp)
            o1 = pools["big"].tile([C, HW], F32, name=f"o1_{n}", tag="bigbuf")
            _conv_image(nc, pools, w1T, xpr, o1, False, stats1, n)
            out1.append(o1)
            if n == 0:
                # w2 prep hides under conv1 of images 1..7
                w2i = _quant_weights(nc, pools, w2_in, identity, ones_row, "w2")
                w2T = _transpose_taps(nc, pools, w2i, identity, FP8, "w2")
            if n == N_A - 1:
                cc1A = _stats_payload_ar(nc, pools, stats1, 0, N_A, K1, "s1A")

        cc1B = _stats_payload_ar(nc, pools, stats1, N_A, BPC, K1, "s1B")
        mean1, rstd1 = _combine_stats(nc, pools, cc1A, cc1B, epst, "bn1")
        sc1, bi1 = _affine_vecs(nc, pools, g1, b1, mean1, rstd1, K1, K1, "1")

        # ---- phase B: act1 + conv2 per image ----
        stats2 = pools["stats"].tile([C, BPC * TPI * 6], F32, name="stats2")
        out2 = []
        cc2A = None
        for n in range(BPC):
            o1 = out1[n]
            a1 = pools["a1"].tile([C, PW], FP8, name=f"a1_{n}", tag="a1")
            if n < 2:
                _zero_halo(nc, a1)
            a1r = a1.rearrange("p (h w) -> p h w", w=WP)
            o1r = o1.rearrange("p (h w) -> p h w", w=W)
            # act1 entirely on DVE (3 fused tensor_scalar passes) so conv2 of
            # image n+1 never waits on the ACT copyout queue:
            #   u = sc1*o1 + bi1 ; w = max(u,0) + 2^23 ; a1 = min(w - 2^23, 15)
            # (image 0 in two row-segments so conv2's first chunk starts early)
            segs = [(0, ROWS_A), (ROWS_A, H)] if n == 0 else [(0, H)]
            for r0, r1 in segs:
                seg = o1[:, r0 * W:r1 * W]
                nc.vector.tensor_scalar(seg, seg, sc1[:], bi1[:],
                                        op0=mybir.AluOpType.mult,
                                        op1=mybir.AluOpType.add)
                nc.vector.tensor_scalar(seg, seg, 0.0, C23,
                                        op0=mybir.AluOpType.max,
                                        op1=mybir.AluOpType.add)
                nc.vector.tensor_scalar(
                    a1r[:, 1 + r0:1 + r1, 1:1 + W], o1r[:, r0:r1, :],
                    C23, 15.0,
                    op0=mybir.AluOpType.subtract,
                    op1=mybir.AluOpType.min)
            o2 = pools["big"].tile([C, HW], F32R, name=f"o2_{n}", tag="bigbuf")
            _conv_image(nc, pools, w2T, a1r, o2, USE_DR, stats2, n)
            out2.append(o2)
            if n == N_A - 1:
                cc2A = _stats_payload_ar(nc, pools, stats2, 0, N_A, K2, "s2A")

        cc2B = _stats_payload_ar(nc, pools, stats2, N_A, BPC, K2, "s2B")
        mean2, rstd2 = _combine_stats(nc, pools, cc2A, cc2B, epst, "bn2")
        sc2, bi2 = _affine_vecs(nc, pools, g2, b2, mean2, rstd2, K1, K2, "2")
        d1 = pools["stats"].tile([C, C], F32R, name="d1")
        nc.vector.tensor_scalar_mul(d1[:], identity[:], sc2[:])
        d2 = pools["stats"].tile([C, C], FP16, name="d2")
        nc.vector.tensor_scalar_mul(d2[:], identity[:], bi2[:])

        # ---- tail: PSUM = d1@o2 + i15@x16 + d2@ones ; rint/clip/scale ----
        d1r = d1[:]
        for n in range(BPC):
            o2 = out2[n]
            o2r = o2[:]
            xpr = xp16s[n].rearrange("p (h w) -> p h w", w=WP)
            for ci, (t0, ntil) in enumerate(CHUNKS):
                pool = pools["psA" if ci == 0 else "psB"]
                ps = pool.tile([C, PSTRIDE * ntil], F32, name=f"tl{ci}",
                               tag=f"cvch{ci}")
                for i in range(ntil):
                    t = t0 + i
                    sl = ps[:, i * PSTRIDE:i * PSTRIDE + TN]
                    nc.tensor.matmul(sl, d1r, o2r[:, t * TN:(t + 1) * TN],
                                     start=True, stop=False)
                    nc.tensor.matmul(sl, i15[:],
                                     xpr[:, RPT * t + 1:RPT * t + 1 + RPT,
                                         1:1 + W],
                                     start=False, stop=False)
                    nc.tensor.matmul(sl, d2[:], ones[:],
                                     start=False, stop=True)
                src = ps.rearrange("p (t c) -> p t c", c=PSTRIDE)[:, :, 0:TN]
                to = pools["stage"].tile([C, TN * ntil], F32, name=f"to{ci}",
                                         tag="stage")
                flat = to[:]
                dst = flat.rearrange("p (t c) -> p t c", c=TN)
                # rint via +2^23 (ACT computes in fp32; add rounds to integer)
                nc.scalar.activation(dst, src,
                                     mybir.ActivationFunctionType.Copy,
                                     bias=C23)
                nc.vector.tensor_scalar(flat, flat, C23, C23 + 15.0,
                                        op0=mybir.AluOpType.max,
                                        op1=mybir.AluOpType.min)
                nc.vector.tensor_scalar(flat, flat, C23, 1.0 / 15.0,
                                        op0=mybir.AluOpType.subtract,
                                        op1=mybir.AluOpType.mult)
                nc.sync.dma_start(
                    out_d[n][:, t0 * RPT:(t0 + ntil) * RPT, :],
                    flat.rearrange("p (h w) -> p h w", w=W))

    nc.compile()
    return nc


def _get_nc():
    if "nc" not in _CACHE:
        _CACHE["nc"] = build()
    return _CACHE["nc"]


def kernel(x, w1, w2, gamma1, beta1, gamma2, beta2, _trace=False):
    nc = _get_nc()
    x = np.ascontiguousarray(np.asarray(x, dtype=np.float32))
    in_common = {
        "w1": np.ascontiguousarray(np.asarray(w1, np.float32).reshape(C, C * 9)),
        "w2": np.ascontiguousarray(np.asarray(w2, np.float32).reshape(C, C * 9)),
        "gamma1": np.asarray(gamma1, np.float32).reshape(C, 1),
        "beta1": np.asarray(beta1, np.float32).reshape(C, 1),
        "gamma2": np.asarray(gamma2, np.float32).reshape(C, 1),
        "beta2": np.asarray(beta2, np.float32).reshape(C, 1),
    }
    in_maps = [dict(in_common, x=x[c * BPC:(c + 1) * BPC]) for c in range(N_CORES)]
    res = bass_utils.run_bass_kernel_spmd(nc, in_maps, core_ids=list(range(N_CORES)),
                                          trace=_trace)
    out = np.concatenate([res.results[c]["out"] for c in range(N_CORES)], axis=0)
    if _trace:
        _CACHE["last_exec_time_ns"] = res.exec_time_ns
        _CACHE["last_results"] = res
    return out


if __name__ == "__main__":
    nc = build()
    print("built ok")


# revision 46
# speedup vs baseline: 1.0891x; 1.0281x over previous
"""Trainium2 Bass kernel for quantized BasicBlock (DoReFa conv-bn-act x2 + residual).

Self-contained: builds an 8-core SPMD Bass kernel, shards the batch (64 -> 8x8),
runs via bass_utils.run_bass_kernel_spmd, gathers the full output.

Math (per core, batch shard of 8 images):
  W_int = 2*rint(tanh(w)*s + 7.5) - 15, s = 15/(2*max|tanh(w)|)   (odd ints, |.|<=15)
  conv1: S1 = conv3x3(fp16(x), W1_int)      == 15 * conv3x3(x, w_q1) + eps_fp16
  BN1 stats of S1 over (N,H,W) all-reduced across cores (split 6+2 images so the
  first AllReduce hides under the tail of conv1)
  act1  = min(rint(relu(S1*sc1 + bi1)), 15)  (ints 0..15, stored fp8e4m3)
  conv2: S2 = conv3x3(act1, W2_int)          == 225 * conv3x3(a_q, w_q2), exact
         (fp8 matmuls; dx-adjacent tap pairs fused via DoubleRow perf mode)
  BN2 stats of S2 all-reduced (same 6+2 split)
  tail : PSUM = diag(15*sc2)@S2 + (15*I)@x + diag(15*bi2)@ones   (f32r matmuls)
         out  = (clip(rint(PSUM), 0, 15)) / 15
"""
import sys
from contextlib import ExitStack

import numpy as np

for _p in ("/opt/trn_rl_repo",):
    if _p not in sys.path:
        sys.path.append(_p)

import concourse.bass as bass
import concourse.bass_isa as bass_isa
import concourse.bacc as bacc
import concourse.mybir as mybir
import concourse.tile as tile
from concourse import bass_utils
from concourse.bass import AP
from concourse.masks import make_identity

F32 = mybir.dt.float32
F32R = mybir.dt.float32r
FP16 = mybir.dt.float16
FP8 = mybir.dt.float8e4

N_CORES = 8
B, C, H, W = 64, 128, 56, 56
BPC = B // N_CORES            # images per core
HP, WP = H + 2, W + 2         # padded 58x58
PW = HP * WP                  # 3364
HW = H * W                    # 3136
RPT = 8                       # output rows per PSUM tile
TN = RPT * W                  # 448 columns per matmul
TPI = H // RPT                # 7 tiles per image
PSTRIDE = 512                 # PSUM bank stride in f32 elements
C23 = float(2 ** 23)
K1 = 15.0                     # conv1 PSUM = 15 * true conv
K2 = 225.0                    # conv2 PSUM = 225 * true conv
N_A = 4                       # images in the first (hidden) stats AllReduce
ROWS_A = 33                   # x rows feeding conv chunk A (+1 halo overlap)
USE_DR = True                 # DoubleRow fp8 pairing for conv2

TAPS = [(dy, dx) for dy in range(3) for dx in range(3)]

# conv PSUM chunks: (first tile, n tiles). 4+3 tiles -> 4+3 banks, 8th bank for
# the weight transposes.
CHUNKS = [(0, 4), (4, 3)]

_CACHE = {}


def _quant_weights(nc, pools, w_in, identity, ones_row, name):
    """DMA + DoReFa-quantize weights in-place on one (C, C*9) f32 tile.

    The cross-partition absmax runs on PE/DVE (transpose -> free-axis reduce
    -> matmul broadcast) instead of gpsimd: the Q7 custom-op launch costs
    ~15us on the critical path.
    """
    wp = pools["wprep"]
    trp = pools["psT"]
    wk = wp.tile([C, C * 9], F32, name=f"{name}_wk", tag="wk")
    half = C * 9 // 2
    nc.scalar.dma_start(wk[:, 0:half], w_in[:, 0:half])
    nc.scalar.dma_start(wk[:, half:], w_in[:, half:])
    am = wp.tile([C, 1], F32, name=f"{name}_am", tag="wam")
    nc.vector.tensor_reduce(am[:], wk[:], axis=mybir.AxisListType.X,
                            op=mybir.AluOpType.max, apply_absolute_value=True)
    nc.scalar.activation(wk[:], wk[:], mybir.ActivationFunctionType.Tanh)
    # partition max: transpose [C,1] -> [1,C], reduce on one lane, broadcast
    psr = trp.tile([C, C], F32, name=f"{name}_psr", tag="trps")
    nc.tensor.transpose(psr[0:1, 0:C], am[:], identity[:])
    amr = wp.tile([1, C], F32, name=f"{name}_amr", tag="wamr")
    nc.scalar.copy(amr[:], psr[0:1, 0:C])
    am0 = wp.tile([1, 1], F32, name=f"{name}_am0", tag="wam0")
    nc.vector.tensor_reduce(am0[:], amr[:], axis=mybir.AxisListType.X,
                            op=mybir.AluOpType.max)
    psb = trp.tile([C, C], F32, name=f"{name}_psb", tag="trps")
    nc.tensor.matmul(psb[0:C, 0:1], ones_row[:], am0[:], start=True, stop=True)
    amg = wp.tile([C, 1], F32, name=f"{name}_amg", tag="wamg")
    nc.scalar.copy(amg[:], psb[0:C, 0:1])
    s_t = wp.tile([C, 1], F32, name=f"{name}_s", tag="ws")
    nc.scalar.activation(s_t[:], amg[:], mybir.ActivationFunctionType.Tanh)
    nc.vector.reciprocal(s_t[:], s_t[:])
    nc.vector.tensor_scalar_mul(s_t[:], s_t[:], 7.5)
    # W_int = 2*rint(tanh*s + 7.5) - 15
    nc.vector.tensor_scalar(wk[:], wk[:], s_t[:], 7.5,
                            op0=mybir.AluOpType.mult, op1=mybir.AluOpType.add)
    nc.vector.tensor_scalar(wk[:], wk[:], C23, C23,
                            op0=mybir.AluOpType.add, op1=mybir.AluOpType.subtract)
    nc.vector.tensor_scalar(wk[:], wk[:], 2.0, 15.0,
                            op0=mybir.AluOpType.mult, op1=mybir.AluOpType.subtract)
    return wk


def _transpose_taps(nc, pools, wint, identity, out_dt, name):
    """Per-tap PE transpose of W_int (O,(I,t)) -> wT (I,(t,O)) in out_dt."""
    wp = pools["wconst"]
    trp = pools["psT"]
    wT = wp.tile([C, 9 * C], out_dt, name=f"{name}_T")
    wr = wint.rearrange("p (i t) -> p i t", t=9)
    for t in range(9):
        ps = trp.tile([C, C], F32, name=f"{name}_ps{t}", tag="trps")
        nc.tensor.transpose(ps[:], wr[:, :, t], identity[:])
        nc.scalar.copy(wT[:, t * C:(t + 1) * C], ps[:])
    return wT


def _warmup_allreduce_eps(nc, pools):
    """Tiny AllReduce at kernel start: warms up ncfw and produces the BN
    epsilon constant (8 * 1e-5/8) so it survives DCE."""
    sp = pools["stats"]
    dp = pools["dram"]
    eps8 = sp.tile([C, 1], F32, name="eps8")
    nc.gpsimd.memset(eps8[:], 1e-5 / N_CORES)
    cc_in = dp.tile([C, 1], F32, name="ccw_in")
    cc_out = dp.tile([C, 1], F32, name="ccw_out")
    nc.gpsimd.dma_start(cc_in[:], eps8[:])
    nc.gpsimd.collective_compute(
        "AllReduce", mybir.AluOpType.add,
        replica_groups=[list(range(N_CORES))],
        ins=[cc_in.opt()], outs=[cc_out.opt()],
    )
    epst = sp.tile([C, 1], F32, name="epst")
    nc.sync.dma_start(epst[:], cc_out[:])
    return epst


def _stats_payload_ar(nc, pools, stats, i0, i1, k_scale, name):
    """bn_aggr over images [i0, i1) -> payload (sum_x, sum_x2)/N_global in
    UNSCALED units -> AllReduce. Returns the DRAM tile holding the result."""
    sp = pools["stats"]
    dp = pools["dram"]
    nimg = i1 - i0
    wfrac = float(nimg) / float(B)
    loc = sp.tile([C, 2], F32, name=f"{name}_loc")
    nc.vector.bn_aggr(loc[:], stats[:, i0 * TPI * 6:i1 * TPI * 6].rearrange(
        "p (t k) -> p t k", k=3))
    mu = sp.tile([C, 1], F32, name=f"{name}_mu")
    nc.vector.tensor_scalar_mul(mu[:], loc[:, 0:1], 1.0 / k_scale)
    pay = sp.tile([C, 2], F32, name=f"{name}_pay")
    nc.vector.tensor_scalar_mul(pay[:, 0:1], mu[:], wfrac)
    # pay1 = wfrac * (var/k^2 + mu^2)
    msq = sp.tile([C, 1], F32, name=f"{name}_msq")
    nc.vector.scalar_tensor_tensor(msq[:], mu[:], wfrac, mu[:],
                                   op0=mybir.AluOpType.mult,
                                   op1=mybir.AluOpType.mult)
    nc.vector.scalar_tensor_tensor(pay[:, 1:2], loc[:, 1:2],
                                   wfrac / (k_scale * k_scale), msq[:],
                                   op0=mybir.AluOpType.mult,
                                   op1=mybir.AluOpType.add)
    cc_in = dp.tile([C, 2], F32, name=f"{name}_in")
    cc_out = dp.tile([C, 2], F32, name=f"{name}_out")
    # scalar queue: its DMA semaphores aren't shared with the x-load stream,
    # so the collective trigger can't get coupled to unrelated transfers
    nc.scalar.dma_start(cc_in[:], pay[:])
    nc.gpsimd.collective_compute(
        "AllReduce", mybir.AluOpType.add,
        replica_groups=[list(range(N_CORES))],
        ins=[cc_in.opt()], outs=[cc_out.opt()],
    )
    return cc_out


def _combine_stats(nc, pools, ccA, ccB, epst, name):
    """Fetch both AllReduce results, combine -> (mean_u, rstd_u)."""
    sp = pools["stats"]
    gA = sp.tile([C, 2], F32, name=f"{name}_gA")
    gB = sp.tile([C, 2], F32, name=f"{name}_gB")
    nc.sync.dma_start(gA[:], ccA[:])
    nc.sync.dma_start(gB[:], ccB[:])
    gs = sp.tile([C, 2], F32, name=f"{name}_gs")
    nc.vector.tensor_tensor(gs[:], gA[:], gB[:], op=mybir.AluOpType.add)
    mean_g = gs[:, 0:1]
    m2 = sp.tile([C, 1], F32, name=f"{name}_m2")
    nc.vector.scalar_tensor_tensor(m2[:], mean_g, 1.0, mean_g,
                                   op0=mybir.AluOpType.mult,
                                   op1=mybir.AluOpType.mult)
    varg = sp.tile([C, 1], F32, name=f"{name}_var")
    nc.vector.scalar_tensor_tensor(varg[:], m2[:], -1.0, gs[:, 1:2],
                                   op0=mybir.AluOpType.mult,
                                   op1=mybir.AluOpType.add)
    std = sp.tile([C, 1], F32, name=f"{name}_std")
    nc.scalar.activation(std[:], varg[:], mybir.ActivationFunctionType.Sqrt,
                         bias=epst[:])
    rstd = sp.tile([C, 1], F32, name=f"{name}_rstd")
    nc.vector.reciprocal(rstd[:], std[:])
    return mean_g, rstd


def _affine_vecs(nc, pools, gamma, beta, mean_u, rstd_u, m_out, k_scale, name):
    """For y_out = m*bn(S/k): sc = m*gamma*rstd/k ; bi = m*(beta - mean_u*gamma*rstd)."""
    sp = pools["stats"]
    gr = sp.tile([C, 1], F32, name=f"gr{name}")
    nc.vector.scalar_tensor_tensor(gr[:], gamma[:], 1.0, rstd_u[:],
                                   op0=mybir.AluOpType.bypass,
                                   op1=mybir.AluOpType.mult)
    sc = sp.tile([C, 1], F32, name=f"sc{name}")
    nc.vector.tensor_scalar_mul(sc[:], gr[:], m_out / k_scale)
    negms = sp.tile([C, 1], F32, name=f"negms{name}")
    nc.vector.scalar_tensor_tensor(negms[:], mean_u, -1.0, gr[:],
                                   op0=mybir.AluOpType.mult,
                                   op1=mybir.AluOpType.mult)
    bi = sp.tile([C, 1], F32, name=f"bi{name}")
    nc.vector.scalar_tensor_tensor(bi[:], negms[:], 1.0, beta[:],
                                   op0=mybir.AluOpType.bypass,
                                   op1=mybir.AluOpType.add)
    nc.vector.tensor_scalar_mul(bi[:], bi[:], m_out)
    return sc, bi


def _dr_rhs(img_view, t, dy):
    """DoubleRow rhs: overlapping 4D AP [C, 2, RPT, W]; pair dim = dx 0/1
    (stride 1 fp8 element)."""
    base = img_view[:, RPT * t + dy: RPT * t + dy + RPT, 0:W]
    u = base.unsqueeze(1)
    ap = [list(p) for p in u.ap]
    ap[1] = [1, 2]
    return AP(u.tensor, u.offset, ap)


def _img_stats(nc, stats, out_sb, n):
    """Per-tile bn_stats for one conv output image."""
    for t in range(TPI):
        gi = n * TPI + t
        nc.vector.bn_stats(stats[:, gi * 6:(gi + 1) * 6],
                           out_sb[:, t * TN:(t + 1) * TN].bitcast(F32))


def _conv_image(nc, pools, wT, img_view, out_sb, use_dr):
    """One image: 2 PSUM chunks; per tile accumulate 9 taps; per chunk a single
    strided ACT copy-out PSUM -> out_sb."""
    for ci, (t0, ntil) in enumerate(CHUNKS):
        pool = pools["psA" if ci == 0 else "psB"]
        ps = pool.tile([C, PSTRIDE * ntil], F32, name=f"cv{ci}",
                       tag=f"cvch{ci}")
        for i in range(ntil):
            t = t0 + i
            sl = ps[:, i * PSTRIDE:i * PSTRIDE + TN]
            if use_dr:
                for dy in range(3):
                    lhsT = wT[:, (3 * dy) * C:(3 * dy + 2) * C].rearrange(
                        "p (two f) -> p two f", two=2)
                    nc.tensor.matmul(sl, lhsT, _dr_rhs(img_view, t, dy),
                                     start=(dy == 0), stop=False,
                                     perf_mode=mybir.MatmulPerfMode.DoubleRow)
                for dy in range(3):
                    k = 3 * dy + 2
                    rhs = img_view[:, RPT * t + dy: RPT * t + dy + RPT, 2:2 + W]
                    nc.tensor.matmul(sl, wT[:, k * C:(k + 1) * C], rhs,
                                     start=False, stop=(dy == 2))
            else:
                for k, (dy, dx) in enumerate(TAPS):
                    rhs = img_view[:, RPT * t + dy: RPT * t + dy + RPT,
                                   dx: dx + W]
                    nc.tensor.matmul(sl, wT[:, k * C:(k + 1) * C], rhs,
                                     start=(k == 0), stop=(k == 8))
        # single strided copy-out for the chunk
        src = ps.rearrange("p (t c) -> p t c", c=PSTRIDE)[:, :, 0:TN]
        dst = out_sb[:, t0 * TN:(t0 + ntil) * TN].rearrange(
            "p (t c) -> p t c", c=TN)
        nc.scalar.copy(dst, src)


def _zero_halo(nc, xb, dt_zero=0.0):
    """Zero the 1-px halo of a padded [C, PW] image tile (3 memsets)."""
    xbr = xb.rearrange("p (h w) -> p h w", w=WP)
    nc.gpsimd.memset(xbr[:, 0, :], dt_zero)
    nc.gpsimd.memset(xbr[:, HP - 1, :], dt_zero)
    side = xb[:, WP - 1:WP - 1 + (HP - 1) * WP].rearrange(
        "p (a b) -> p a b", b=WP)
    nc.gpsimd.memset(side[:, :, 0:2], dt_zero)


def build():
    nc = bacc.Bacc("TRN2", target_bir_lowering=False, debug=False,
                   enable_asserts=False, num_devices=N_CORES)
    x_in = nc.dram_tensor("x", [BPC, C, H, W], F32, kind="ExternalInput").ap()
    w1_in = nc.dram_tensor("w1", [C, C * 9], F32, kind="ExternalInput").ap()
    w2_in = nc.dram_tensor("w2", [C, C * 9], F32, kind="ExternalInput").ap()
    g1_in = nc.dram_tensor("gamma1", [C, 1], F32, kind="ExternalInput").ap()
    b1_in = nc.dram_tensor("beta1", [C, 1], F32, kind="ExternalInput").ap()
    g2_in = nc.dram_tensor("gamma2", [C, 1], F32, kind="ExternalInput").ap()
    b2_in = nc.dram_tensor("beta2", [C, 1], F32, kind="ExternalInput").ap()
    out_d = nc.dram_tensor("out", [BPC, C, H, W], F32, kind="ExternalOutput").ap()

    with tile.TileContext(nc) as tc, ExitStack() as ctx:
        pools = {
            "wprep": ctx.enter_context(tc.tile_pool(name="wprep", bufs=1)),
            "wconst": ctx.enter_context(tc.tile_pool(name="wconst", bufs=1)),
            "stats": ctx.enter_context(tc.tile_pool(name="stats", bufs=1)),
            "xp16": ctx.enter_context(tc.tile_pool(name="xp16", bufs=8)),
            "big": ctx.enter_context(tc.tile_pool(name="big", bufs=8)),
            "a1": ctx.enter_context(tc.tile_pool(name="a1", bufs=2)),
            # shared staging ring: weight-quant scratch, x fp32 staging, and
            # tail result buffers all rotate through 3 slots
            "stage": ctx.enter_context(tc.tile_pool(name="stage", bufs=3)),
            "psA": ctx.enter_context(
                tc.tile_pool(name="psA", bufs=1, space="PSUM")),
            "psB": ctx.enter_context(
                tc.tile_pool(name="psB", bufs=1, space="PSUM")),
            "psT": ctx.enter_context(
                tc.tile_pool(name="psT", bufs=1, space="PSUM")),
            "dram": ctx.enter_context(tc.tile_pool(name="dram", bufs=12,
                                                   space="DRAM")),
        }
        consts = pools["wconst"]

        # per-channel params (sync queue; scalar queue is loading w1)
        g1 = consts.tile([C, 1], F32, name="g1"); nc.sync.dma_start(g1[:], g1_in[:])
        b1 = consts.tile([C, 1], F32, name="b1"); nc.sync.dma_start(b1[:], b1_in[:])
        g2 = consts.tile([C, 1], F32, name="g2"); nc.sync.dma_start(g2[:], g2_in[:])
        b2 = consts.tile([C, 1], F32, name="b2"); nc.sync.dma_start(b2[:], b2_in[:])

        identity = consts.tile([C, C], F32, name="identity")
        make_identity(nc, identity[:])
        # fp16 identity*15 and ones for the tail matmuls (fp16 weights keep
        # FWL weight loads fast); the diag(sc2) stays f32r for precision
        i15 = consts.tile([C, C], FP16, name="i15")
        nc.vector.tensor_scalar_mul(i15[:], identity[:], 15.0)
        ones = consts.tile([C, TN], FP16, name="ones")
        nc.vector.memset(ones[:], 1.0)
        ones_row = consts.tile([1, C], F32, name="ones_row")
        nc.vector.memset(ones_row[:], 1.0)

        epst = _warmup_allreduce_eps(nc, pools)

        # ---- w1 quant + transpose (critical path to first conv MM) ----
        w1i = _quant_weights(nc, pools, w1_in, identity, ones_row, "w1")
        w1T = _transpose_taps(nc, pools, w1i, identity, FP16, "w1")

        # ---- phase A: conv1 per image (single fp16 pass) ----
        stats1 = pools["stats"].tile([C, BPC * TPI * 6], F32, name="stats1")
        out1 = []
        cc1A = None
        w2T = None
        xp16s = []
        for n in range(BPC):
            # x staged fp32 (sync DMA, two halves) then ACT-converted into the
            # padded fp16 image; conv chunk A only depends on the first 33
            # rows (region-granular deps). The fp16 copy also serves as the
            # tail's residual (no reload).
            xin = x_in[n].rearrange("c h w -> c (h w)")
            xp = pools["xp16"].tile([C, PW], FP16, name=f"xp{n}", tag="xp")
            _zero_halo(nc, xp)
            xpr = xp.rearrange("p (h w) -> p h w", w=WP)
            # converts run on DVE so they never queue behind the ACT copyouts
            xsA = pools["stage"].tile([C, ROWS_A * W], F32, name=f"xsA{n}",
                                      tag="stage")
            nc.sync.dma_start(xsA[:], xin[:, 0:ROWS_A * W])
            nc.vector.tensor_copy(xpr[:, 1:1 + ROWS_A, 1:1 + W],
                                  xsA.rearrange("p (h w) -> p h w", w=W))
            xsB = pools["stage"].tile([C, (H - ROWS_A) * W], F32,
                                      name=f"xsB{n}", tag="stage")
            nc.sync.dma_start(xsB[:], xin[:, ROWS_A * W:])
            nc.vector.tensor_copy(xpr[:, 1 + ROWS_A:1 + H, 1:1 + W],
                                  xsB.rearrange("p (h w) -> p h w", w=W))
            xp16s.append(xp)
            # stats for image n-1 emitted here so image n's x-prep and conv
            # never queue behind them on DVE
            if n >= 1:
                _img_stats(nc, stats1, out1[n - 1], n - 1)
                if n - 1 == N_A - 1:
                    cc1A = _stats_payload_ar(nc, pools, stats1, 0, N_A, K1,
                                             "s1A")
            o1 = pools["big"].tile([C, HW], F32, name=f"o1_{n}", tag="bigbuf")
            _conv_image(nc, pools, w1T, xpr, o1, False)
            out1.append(o1)
            if n == 0:
                # w2 prep hides under conv1 of images 1..7
                w2i = _quant_weights(nc, pools, w2_in, identity, ones_row, "w2")
                w2T = _transpose_taps(nc, pools, w2i, identity, FP8, "w2")

        _img_stats(nc, stats1, out1[BPC - 1], BPC - 1)
        cc1B = _stats_payload_ar(nc, pools, stats1, N_A, BPC, K1, "s1B")
        mean1, rstd1 = _combine_stats(nc, pools, cc1A, cc1B, epst, "bn1")
        sc1, bi1 = _affine_vecs(nc, pools, g1, b1, mean1, rstd1, K1, K1, "1")

        # ---- phase B: act1 + conv2 per image ----
        stats2 = pools["stats"].tile([C, BPC * TPI * 6], F32, name="stats2")
        out2 = []
        cc2A = None
        for n in range(BPC):
            o1 = out1[n]
            a1 = pools["a1"].tile([C, PW], FP8, name=f"a1_{n}", tag="a1")
            if n < 2:
                _zero_halo(nc, a1)
            a1r = a1.rearrange("p (h w) -> p h w", w=WP)
            o1r = o1.rearrange("p (h w) -> p h w", w=W)
            # act1 entirely on DVE (3 fused tensor_scalar passes) so conv2 of
            # image n+1 never waits on the ACT copyout queue:
            #   u = sc1*o1 + bi1 ; w = max(u,0) + 2^23 ; a1 = min(w - 2^23, 15)
            # (image 0 in two row-segments so conv2's first chunk starts early)
            segs = [(0, ROWS_A), (ROWS_A, H)] if n == 0 else [(0, H)]
            for r0, r1 in segs:
                seg = o1[:, r0 * W:r1 * W]
                nc.vector.tensor_scalar(seg, seg, sc1[:], bi1[:],
                                        op0=mybir.AluOpType.mult,
                                        op1=mybir.AluOpType.add)
                nc.vector.tensor_scalar(seg, seg, 0.0, C23,
                                        op0=mybir.AluOpType.max,
                                        op1=mybir.AluOpType.add)
                nc.vector.tensor_scalar(
                    a1r[:, 1 + r0:1 + r1, 1:1 + W], o1r[:, r0:r1, :],
                    C23, 15.0,
                    op0=mybir.AluOpType.subtract,
                    op1=mybir.AluOpType.min)
            if n >= 1:
                _img_stats(nc, stats2, out2[n - 1], n - 1)
                if n - 1 == N_A - 1:
                    cc2A = _stats_payload_ar(nc, pools, stats2, 0, N_A, K2,
                                             "s2A")
            o2 = pools["big"].tile([C, HW], F32R, name=f"o2_{n}", tag="bigbuf")
            _conv_image(nc, pools, w2T, a1r, o2, USE_DR)
            out2.append(o2)

        _img_stats(nc, stats2, out2[BPC - 1], BPC - 1)
        cc2B = _stats_payload_ar(nc, pools, stats2, N_A, BPC, K2, "s2B")
        mean2, rstd2 = _combine_stats(nc, pools, cc2A, cc2B, epst, "bn2")
        sc2, bi2 = _affine_vecs(nc, pools, g2, b2, mean2, rstd2, K1, K2, "2")
        d1 = pools["stats"].tile([C, C], F32R, name="d1")
        nc.vector.tensor_scalar_mul(d1[:], identity[:], sc2[:])
        d2 = pools["stats"].tile([C, C], FP16, name="d2")
        nc.vector.tensor_scalar_mul(d2[:], identity[:], bi2[:])

        # ---- tail: PSUM = d1@o2 + i15@x16 + d2@ones ; rint/clip/scale ----
        d1r = d1[:]
        for n in range(BPC):
            o2 = out2[n]
            o2r = o2[:]
            xpr = xp16s[n].rearrange("p (h w) -> p h w", w=WP)
            for ci, (t0, ntil) in enumerate(CHUNKS):
                pool = pools["psA" if ci == 0 else "psB"]
                ps = pool.tile([C, PSTRIDE * ntil], F32, name=f"tl{ci}",
                               tag=f"cvch{ci}")
                for i in range(ntil):
                    t = t0 + i
                    sl = ps[:, i * PSTRIDE:i * PSTRIDE + TN]
                    nc.tensor.matmul(sl, d1r, o2r[:, t * TN:(t + 1) * TN],
                                     start=True, stop=False)
                    nc.tensor.matmul(sl, i15[:],
                                     xpr[:, RPT * t + 1:RPT * t + 1 + RPT,
                                         1:1 + W],
                                     start=False, stop=False)
                    nc.tensor.matmul(sl, d2[:], ones[:],
                                     start=False, stop=True)
                src = ps.rearrange("p (t c) -> p t c", c=PSTRIDE)[:, :, 0:TN]
                to = pools["stage"].tile([C, TN * ntil], F32, name=f"to{ci}",
                                         tag="stage")
                flat = to[:]
                dst = flat.rearrange("p (t c) -> p t c", c=TN)
                # rint via +2^23 (ACT computes in fp32; add rounds to integer)
                nc.scalar.activation(dst, src,
                                     mybir.ActivationFunctionType.Copy,
                                     bias=C23)
                nc.vector.tensor_scalar(flat, flat, C23, C23 + 15.0,
                                        op0=mybir.AluOpType.max,
                                        op1=mybir.AluOpType.min)
                nc.vector.tensor_scalar(flat, flat, C23, 1.0 / 15.0,
                                        op0=mybir.AluOpType.subtract,
                                        op1=mybir.AluOpType.mult)
                nc.sync.dma_start(
                    out_d[n][:, t0 * RPT:(t0 + ntil) * RPT, :],
                    flat.rearrange("p (h w) -> p h w", w=W))

    nc.compile()
    return nc


def _get_nc():
    if "nc" not in _CACHE:
        _CACHE["nc"] = build()
    return _CACHE["nc"]


def kernel(x, w1, w2, gamma1, beta1, gamma2, beta2, _trace=False):
    nc = _get_nc()
    x = np.ascontiguousarray(np.asarray(x, dtype=np.float32))
    in_common = {
        "w1": np.ascontiguousarray(np.asarray(w1, np.float32).reshape(C, C * 9)),
        "w2": np.ascontiguousarray(np.asarray(w2, np.float32).reshape(C, C * 9)),
        "gamma1": np.asarray(gamma1, np.float32).reshape(C, 1),
        "beta1": np.asarray(beta1, np.float32).reshape(C, 1),
        "gamma2": np.asarray(gamma2, np.float32).reshape(C, 1),
        "beta2": np.asarray(beta2, np.float32).reshape(C, 1),
    }
    in_maps = [dict(in_common, x=x[c * BPC:(c + 1) * BPC]) for c in range(N_CORES)]
    res = bass_utils.run_bass_kernel_spmd(nc, in_maps, core_ids=list(range(N_CORES)),
                                          trace=_trace)
    out = np.concatenate([res.results[c]["out"] for c in range(N_CORES)], axis=0)
    if _trace:
        _CACHE["last_exec_time_ns"] = res.exec_time_ns
        _CACHE["last_results"] = res
    return out


if __name__ == "__main__":
    nc = build()
    print("built ok")
